# revision 13
# baseline (speedup 1.0000x reference)
"""2-layer GAT on 8 Trainium2 NeuronCores (Bass/Tile).

Sharding: the 391 dst 128-node blocks are sorted by half-A edge count and
dealt in groups of 8 to the cores (one block per core per iteration), so the
per-iteration cross-core tile maximum stays near the mean.  Edges are routed
to the core owning their dst block and laid out in shared tiles: half-A rows
(table rows < SPLIT, int16-indexable) first, padded to the iteration max,
then half-B rows in the same tile array (gather B runs first with leading
dummy indices, gather A then overwrites its region).

Per-layer device program (phase B only; projections are fused elsewhere):
  per 128-dst block: dma_gather B + A from the HBM row table
  [h | asrc f32-bits], one-hot dst masks on DVE (pair-packed 2x),
  per-edge adst via maskT matmuls (PSUM-resident), w = exp(prelu(asrc+adst))
  on ACT, weighted rows on DVE, aggregation + softmax denominator via
  PSUM-accumulated matmuls, epilogue scaling on ACT.

Launch 0 projects [h | asrc | adst] per node; layer 1's epilogue fuses the
layer-2 projection [h2 | asrc2 | adst2], so neither layer loads x at all.
Shards are exchanged through the host between launches.
"""

import os
import numpy as np
import ml_dtypes

import concourse.bass as bass
import concourse.bacc as bacc
import concourse.tile as tile
from concourse import mybir
from concourse.bass_utils import run_bass_kernel_spmd

BF16 = ml_dtypes.bfloat16

N = 50000
E = 800000
IN = 128
H1 = 4
F1 = 64
NEG = 0.2
P = 128
NCORES = 8
NB = 49                 # block iterations per core
SHARD = NB * P          # 6272 rows per core in the table
NPAD = 391 * P          # 50048 padded node count
NGB = NCORES * NB       # 392 block slots (391 real + 1 dummy)
SPLIT = 196 * P         # 25088: gather-table half boundary (int16 idx limit)
GRP = 16                # proj-launch load group

_prog_cache = {}


# ----------------------------------------------------------------------------
# host-side edge preprocessing (shared by both layers)
# ----------------------------------------------------------------------------

def _prep_edges(edge_index):
    src = np.concatenate([edge_index[0].astype(np.int64), np.arange(N, dtype=np.int64)])
    dst = np.concatenate([edge_index[1].astype(np.int64), np.arange(N, dtype=np.int64)])
    order = np.argsort(dst, kind="stable")
    s = src[order]
    d = dst[order]

    gb = d >> 7                                   # global 128-block of dst
    cnt = np.bincount(gb, minlength=NGB)
    starts = np.concatenate([[0], np.cumsum(cnt)])
    isB = s >= SPLIT
    cntA = np.zeros(NGB, np.int64)
    for g in range(NGB):
        cntA[g] = np.count_nonzero(~isB[starts[g]:starts[g + 1]])

    # deal blocks sorted by half-A count: iteration i gets ranks [8i, 8i+8)
    blk_order = np.argsort(-cntA, kind="stable")
    asg = blk_order.reshape(NB, NCORES)           # [iter, core] -> global block
    nA = cntA[asg]                                # [NB, NCORES]
    nBc = (cnt - cntA)[asg]
    nAmax = nA.max(1)                             # [NB]
    nBmax = nBc.max(1)
    Tm = np.maximum(-(-(nAmax + nBmax) // P), 1).astype(np.int64)
    niA16 = (-(-nAmax // 16) * 16).astype(np.int64)   # static gather-A num_idxs
    fA = nAmax // P                               # full A tiles
    rA = nAmax % P                                # B's leading dummy count
    niB = (Tm - fA) * P                           # static gather-B num_idxs

    toff = np.zeros(NB + 1, np.int64)
    np.cumsum(Tm, out=toff[1:])
    Ttot = int(toff[NB])
    TMX = int(Tm.max())

    scol = np.zeros(NB + 1, np.int64)             # idx column offsets (per 16)
    np.cumsum(niA16 // 16 + niB // 16, out=scol[1:])
    Stot = int(scol[NB])

    idx_all = np.zeros((NCORES, P, Stot), np.int16)
    dstl = np.full((NCORES, Ttot, P), -1.0, np.float32)   # [t, p] layout

    for i in range(NB):
        sA = int(scol[i])
        sB = sA + int(niA16[i] // 16)
        for m in range(NCORES):
            g = asg[i, m]
            e0, e1 = starts[g], starts[g + 1]
            sb = s[e0:e1]
            mB = isB[e0:e1]
            shA = sb[~mB]
            shB = sb[mB] - SPLIT
            dlA = (d[e0:e1][~mB] - (g << 7)).astype(np.float32)
            dlB = (d[e0:e1][mB] - (g << 7)).astype(np.float32)
            na, nb_ = len(shA), len(shB)
            # gather-A idx: real | dummy-0 to nAmax | -1 tail to niA16
            ia = np.zeros(int(niA16[i]), np.int16)
            ia[:na] = shA
            ia[int(nAmax[i]):] = -1
            # gather-B idx: rA dummy-0 | real | dummy-0 tail
            ib = np.zeros(int(niB[i]), np.int16)
            ib[int(rA[i]):int(rA[i]) + nb_] = shB
            for seg, off in ((ia, sA), (ib, sB)):
                w = seg.reshape(-1, 16).T          # [16, S]
                idx_all[m][:, off:off + w.shape[1]] = np.tile(w, (8, 1))
            dl = np.full(int(Tm[i]) * P, -1.0, np.float32)
            dl[:na] = dlA
            dl[int(nAmax[i]):int(nAmax[i]) + nb_] = dlB
            dstl[m][toff[i]:toff[i] + Tm[i], :] = dl.reshape(int(Tm[i]), P)

    dstl_pt = np.ascontiguousarray(dstl.transpose(0, 2, 1))   # [m, P, Ttot]
    dstl_row = np.full((NCORES, NB, TMX * P), -1.0, np.float32)
    for m in range(NCORES):
        for i in range(NB):
            T = int(Tm[i])
            dstl_row[m, i, :T * P] = dstl[m, toff[i]:toff[i] + T].reshape(-1)
    dstl_row = dstl_row.astype(BF16)

    meta = dict(Tm=Tm.tolist(), toff=toff.tolist(), fA=fA.tolist(),
                rA=rA.tolist(), niA16=niA16.tolist(), niB=niB.tolist(),
                scol=scol.tolist(), Ttot=Ttot, Stot=Stot, Tmax=TMX,
                asg=asg.tolist())
    return meta, idx_all, dstl_pt, dstl_row


# ----------------------------------------------------------------------------
# launch 0: project own shard -> [h | asrc | adst] table slice
# ----------------------------------------------------------------------------

def _build_proj():
    dt = mybir.dt
    KCH, H = 1, H1
    COUT = H1 * F1
    RC = COUT + 2 * H                   # [h | asrc | adst]
    OCOL = COUT + 4 * H                 # bf16 slots: h | asrc bits | adst bits
    nc = bacc.Bacc("TRN2", target_bir_lowering=False, debug=False,
                   num_devices=NCORES)
    xs = nc.dram_tensor("xs", [KCH, P, NB, P], dt.bfloat16,
                        kind="ExternalInput")
    wr = nc.dram_tensor("wr", [KCH, P, RC], dt.bfloat16,
                        kind="ExternalInput")
    hts = nc.dram_tensor("hts", [SHARD, OCOL], dt.bfloat16,
                         kind="ExternalOutput")
    with tile.TileContext(nc) as tc:
        with (
            tc.tile_pool(name="const", bufs=1) as cp,
            tc.tile_pool(name="pa", bufs=3) as pa,
            tc.tile_pool(name="psA", bufs=3, space="PSUM") as psA,
        ):
            wr_sb = cp.tile([P, KCH, RC], dt.bfloat16)
            nc.sync.dma_start(wr_sb[:], wr[:].rearrange("k p c -> p k c"))
            for g0 in range(0, NB, GRP):
                gn = min(GRP, NB - g0)
                xa = pa.tile([P, KCH, gn, P], dt.bfloat16, tag="xa")
                nc.sync.dma_start(
                    xa[:], xs[:, :, g0:g0 + gn, :].rearrange(
                        "k f t n -> f k t n"))
                hst = pa.tile([P, gn, OCOL], dt.bfloat16, tag="hst")
                for t0 in range(0, gn, 2):
                    pn = min(2, gn - t0)
                    ps = psA.tile([P, 2, 512], dt.float32, tag="psa")
                    for t2 in range(pn):
                        for k in range(KCH):
                            nc.tensor.matmul(ps[:, t2, 0:RC],
                                             lhsT=xa[:, k, t0 + t2, :],
                                             rhs=wr_sb[:, k, 0:RC],
                                             start=(k == 0),
                                             stop=(k == KCH - 1))
                    nc.scalar.activation(
                        hst[:, t0:t0 + pn, 0:COUT], ps[:, 0:pn, 0:COUT],
                        mybir.ActivationFunctionType.Copy)
                    nc.vector.tensor_copy(
                        hst[:, t0:t0 + pn, COUT:OCOL].bitcast(dt.float32),
                        ps[:, 0:pn, COUT:COUT + 2 * H])
                nc.sync.dma_start(
                    hts[g0 * P:(g0 + gn) * P, :].rearrange(
                        "(t n) c -> n t c", t=gn),
                    hst[:])
    nc.compile()
    return nc


# ----------------------------------------------------------------------------
# per-layer message-passing program (phase B)
# ----------------------------------------------------------------------------

def _build_layer(meta, layer, zero_bias):
    """layer 1: heads 4, F 64, fused layer-2 row production, no dense out.
    layer 2: heads 1, F 64, out f32 [SHARD, 64]."""
    dt = mybir.dt
    Tm, toff, fAm = meta["Tm"], meta["toff"], meta["fA"]
    niA16, niB, scol = meta["niA16"], meta["niB"], meta["scol"]
    Ttot, Stot, Tmax = meta["Ttot"], meta["Stot"], meta["Tmax"]

    if layer == 1:
        H, F = H1, F1
    else:
        H, F = 1, F1
    COUT = H * F
    AGC = COUT + H                    # aggregation psum cols: [num | den]
    TABC = 384 if layer == 1 else 128  # table row slots (256B granules)

    nc = bacc.Bacc("TRN2", target_bir_lowering=False, debug=False,
                   num_devices=NCORES)

    if layer == 1:
        w2r = nc.dram_tensor("w2r", [2, P, 66], dt.bfloat16,
                             kind="ExternalInput")
        identT = nc.dram_tensor("identT", [P, P], dt.bfloat16,
                                kind="ExternalInput")
        outT2 = nc.dram_tensor("outT2", [SHARD, 68], dt.bfloat16,
                               kind="ExternalOutput")
    else:
        outT = nc.dram_tensor("outT", [SHARD, COUT], dt.float32,
                              kind="ExternalOutput")
    idxT = nc.dram_tensor("idxT", [P, Stot], dt.int16, kind="ExternalInput")
    dstlT = nc.dram_tensor("dstlT", [P, Ttot], dt.bfloat16, kind="ExternalInput")
    dstlR = nc.dram_tensor("dstlR", [NB, Tmax * P], dt.bfloat16,
                           kind="ExternalInput")
    adstT = nc.dram_tensor("adstT", [P, NB * H], dt.bfloat16,
                           kind="ExternalInput")
    brow = nc.dram_tensor("brow", [1, COUT], dt.float32, kind="ExternalInput")
    iot_r = nc.dram_tensor("iot_r", [1, P], dt.bfloat16, kind="ExternalInput")
    iot_c = nc.dram_tensor("iot_c", [P, 1], dt.float32, kind="ExternalInput")
    htab = nc.dram_tensor("htab", [NPAD, TABC], dt.bfloat16,
                          kind="ExternalInput")

    SP = bool(int(os.environ.get("GAT_SP", "0")))
    PBB = int(os.environ.get("GAT_PBB", "4"))
    PPK = int(os.environ.get("GAT_PPK", "3"))
    PBM = int(os.environ.get("GAT_PB_MOD", "2"))

    with tile.TileContext(nc) as tc:
        with (
            tc.tile_pool(name="const", bufs=1) as cp,
            tc.tile_pool(name="keep", bufs=1) as kp,
            tc.tile_pool(name="pp", bufs=3) as ppool,
            tc.tile_pool(name="pb", bufs=PBB) as pb,
            tc.tile_pool(name="sm", bufs=3) as sm,
            tc.tile_pool(name="psA", bufs=1, space="PSUM") as psA,
            tc.tile_pool(name="psB", bufs=2, space="PSUM") as psB,
            tc.tile_pool(name="psD", bufs=PPK + 1, space="PSUM") as psD,
        ):
            # ---- resident constants ----
            b_sb = cp.tile([P, COUT], dt.float32)
            nc.sync.dma_start(b_sb[:], brow[:].broadcast_to([P, COUT]))
            ior_sb = cp.tile([P, P], dt.bfloat16)
            nc.sync.dma_start(ior_sb[:], iot_r[:].broadcast_to([P, P]))
            ioc_sb = cp.tile([P, 1], dt.float32)
            nc.sync.dma_start(ioc_sb[:], iot_c[:])
            if layer == 1:
                w2_sb = cp.tile([P, 2, 66], dt.bfloat16)
                nc.sync.dma_start(w2_sb[:], w2r[:].rearrange("k p c -> p k c"))
                id_sb = cp.tile([P, P], dt.bfloat16)
                nc.sync.dma_start(id_sb[:], identT[:])
            idx_sb = kp.tile([P, Stot], dt.int16)
            nc.sync.dma_start(idx_sb[:], idxT[:])
            dstl_sb = kp.tile([P, Ttot], dt.bfloat16)
            nc.sync.dma_start(dstl_sb[:], dstlT[:])
            adst_sh = kp.tile([P, NB * H], dt.bfloat16)
            nc.sync.dma_start(adst_sh[:], adstT[:])
            if layer == 1:
                oall = kp.tile([P, NB, 68], dt.bfloat16)
            else:
                oall = kp.tile([P, NB, COUT], dt.float32)

            # ---- pre-pass: expand adst to per-edge values (PSUM-resident),
            # staged so the dlr broadcast has a full iteration to land ----
            adst_ps = [None] * NB
            dlr_sb = [None] * NB

            def dlr_issue(b):
                T = Tm[b]
                dlr = ppool.tile([P, T * P], dt.bfloat16, tag="dlr")
                if PBM and b % PBM:
                    dlrow = ppool.tile([1, T * P], dt.bfloat16, tag="dlrow")
                    nc.sync.dma_start(dlrow[:], dstlR[b:b + 1, 0:T * P])
                    nc.gpsimd.partition_broadcast(dlr[:], dlrow[:])
                else:
                    nc.sync.dma_start(
                        dlr[:],
                        dstlR[b:b + 1, 0:T * P].broadcast_to([P, T * P]))
                dlr_sb[b] = dlr

            def prepass_block(b):
                T = Tm[b]
                mT = ppool.tile([P, T, P], dt.bfloat16, tag="mT")
                nc.vector.tensor_scalar(
                    mT[:].rearrange("p t e -> p (t e)"), dlr_sb[b][:],
                    ioc_sb[:], None, mybir.AluOpType.is_equal)
                dlr_sb[b] = None
                ap_ps = psD.tile([P, T * H], dt.float32, tag="adps")
                for t in range(T):
                    nc.tensor.matmul(ap_ps[:, t * H:(t + 1) * H],
                                     lhsT=mT[:, t, :],
                                     rhs=adst_sh[:, b * H:(b + 1) * H],
                                     start=True, stop=True)
                adst_ps[b] = ap_ps

            # ---- gathers: B first (covers tail incl. boundary dummies),
            # then A overwrites its region ----
            htabA = htab[0:SPLIT, :]
            htabB = htab[SPLIT:NPAD, :]

            g_sb = [None] * NB
            mk_sb = [None] * NB

            def issue_gatherB(b):
                T = Tm[b]
                g = pb.tile([P, T, TABC], dt.bfloat16, tag="gath")
                sB = scol[b] + niA16[b] // 16
                if niB[b] > 0:
                    nc.gpsimd.dma_gather(
                        g[:, fAm[b]:T, :], htabB,
                        idx_sb[:, sB:sB + niB[b] // 16],
                        niB[b], niB[b], TABC, single_packet=SP)
                g_sb[b] = g

            def issue_gatherA(b):
                T = Tm[b]
                g = g_sb[b]
                sA = scol[b]
                if niA16[b] > 0:
                    a_tiles = -(-niA16[b] // P)
                    nc.gpsimd.dma_gather(
                        g[:, 0:a_tiles, :], htabA,
                        idx_sb[:, sA:sA + niA16[b] // 16],
                        niA16[b], niA16[b], TABC, single_packet=SP)

            def build_masks(b):
                T = Tm[b]
                # dst one-hot masks (pair-packed for DVE 2x)
                dl2 = sm.tile([P, T, 2], dt.bfloat16, tag="dl2")
                nc.vector.tensor_copy(
                    dl2[:],
                    dstl_sb[:, toff[b]:toff[b] + T].rearrange(
                        "p (t o) -> p t o", o=1).broadcast_to([P, T, 2]))
                mk = sm.tile([P, T, P], dt.bfloat16, tag="mk")   # [e_p,(t,d)]
                nc.vector.tensor_tensor(
                    mk[:].rearrange("p t (d2 pr) -> p t d2 pr", pr=2),
                    ior_sb[:].rearrange("p (t d2 pr) -> p t d2 pr", t=1, pr=2
                                        ).broadcast_to([P, T, P // 2, 2]),
                    dl2[:].rearrange("p t (d2 pr) -> p t d2 pr", d2=1
                                     ).broadcast_to([P, T, P // 2, 2]),
                    mybir.AluOpType.is_equal)
                mk_sb[b] = mk

            def epilogue(b, agg):
                # out = num/(den+eps) (+bias) (+ELU and fused proj, layer 1)
                dn = sm.tile([P, H], dt.float32, tag="dn")
                nc.vector.tensor_scalar_add(dn[:], agg[:, COUT:AGC], 1e-16)
                rc = sm.tile([P, H], dt.float32, tag="rc")
                nc.vector.reciprocal(rc[:], dn[:])
                if layer == 1:
                    ob = sm.tile([P, COUT], dt.bfloat16, tag="ob")
                    for h in range(H):
                        nc.scalar.activation(ob[:, h * F:(h + 1) * F],
                                             agg[:, h * F:(h + 1) * F],
                                             mybir.ActivationFunctionType.Copy,
                                             scale=rc[:, h:h + 1])
                    if not zero_bias:
                        nc.vector.tensor_add(
                            ob[:], ob[:],
                            b_sb[:].bitcast(dt.bfloat16)[:, 1::2])
                    # elu(y) = relu(y) + exp(min(y,0)) - 1
                    r1 = sm.tile([P, COUT], dt.bfloat16, tag="r1")
                    nc.scalar.activation(r1[:], ob[:],
                                         mybir.ActivationFunctionType.Relu,
                                         scale=-1.0)
                    r2 = sm.tile([P, COUT], dt.bfloat16, tag="r2")
                    nc.scalar.activation(r2[:], r1[:],
                                         mybir.ActivationFunctionType.Exp,
                                         scale=-1.0)
                    nc.scalar.activation(ob[:], ob[:],
                                         mybir.ActivationFunctionType.Relu)
                    nc.vector.scalar_tensor_tensor(
                        ob[:], r2[:], -1.0, ob[:],
                        mybir.AluOpType.add, mybir.AluOpType.add)
                    # fused layer-2 row production:
                    # [elu(out1) @ [W2|wasrc2|wadst2]] -> [h2|asrc2|adst2]
                    ps_t = psA.tile([P, 2, P], dt.bfloat16, tag="pst")
                    for c in range(2):
                        nc.tensor.transpose(ps_t[:, c, :],
                                            ob[:, c * P:(c + 1) * P],
                                            id_sb[:])
                    x2T = sm.tile([P, 2, P], dt.bfloat16, tag="x2T")
                    nc.scalar.activation(x2T[:], ps_t[:],
                                         mybir.ActivationFunctionType.Copy)
                    ps2 = psA.tile([P, 66], dt.float32, tag="ps2")
                    for c in range(2):
                        nc.tensor.matmul(ps2[:], lhsT=x2T[:, c, :],
                                         rhs=w2_sb[:, c, :],
                                         start=(c == 0), stop=(c == 1))
                    nc.scalar.activation(oall[:, b, 0:64], ps2[:, 0:64],
                                         mybir.ActivationFunctionType.Copy)
                    nc.vector.tensor_copy(
                        oall[:, b, 64:68].bitcast(dt.float32), ps2[:, 64:66])
                else:
                    nc.scalar.activation(oall[:, b, :], agg[:, 0:COUT],
                                         mybir.ActivationFunctionType.Copy,
                                         scale=rc[:, 0:1])
                    if not zero_bias:
                        nc.vector.tensor_add(oall[:, b, :], oall[:, b, :],
                                             b_sb[:])

            # ---- phase B: software-pipelined per-block message passing.
            # Emission order is tuned for the in-order engine queues: the
            # et->prelu->exp->hp critical chain leads, lookahead issues fill
            # the ACT round-trip, the lagged epilogue never blocks it. ----
            for q in range(min(PPK, NB)):
                dlr_issue(q)
            for q in range(min(PPK - 1, NB)):
                prepass_block(q)
            for q in range(min(2, NB)):
                issue_gatherB(q)
            issue_gatherA(0)
            build_masks(0)
            pend = None                     # (block, agg) awaiting epilogue
            for b in range(NB):
                T = Tm[b]
                g = g_sb[b]
                mk = mk_sb[b]

                # w2 = exp(prelu(asrc + adst)) pair-broadcast, on ACT
                et = sm.tile([P, T * H], dt.float32, tag="et")
                nc.vector.tensor_tensor(
                    et[:].rearrange("p (t h) -> p t h", h=H),
                    g[:, :, COUT:COUT + 2 * H].bitcast(dt.float32),
                    adst_ps[b][:].rearrange("p (t h) -> p t h", h=H),
                    mybir.AluOpType.add)
                adst_ps[b] = None
                lr = sm.tile([P, T * H], dt.float32, tag="lr")
                nc.scalar.activation(lr[:], et[:],
                                     mybir.ActivationFunctionType.Prelu,
                                     alpha=NEG)
                wt2 = sm.tile([P, T, H, 2], dt.bfloat16, tag="wt2")
                nc.scalar.activation(
                    wt2[:],
                    lr[:].rearrange("p (t h o) -> p t h o", h=H, o=1
                                    ).broadcast_to([P, T, H, 2]),
                    mybir.ActivationFunctionType.Exp)

                # lookahead issues (fill the ACT round-trip gap on DVE/Pool;
                # the critical gather A goes first in the DMA queue)
                if b + 1 < NB:
                    issue_gatherA(b + 1)
                if b + PPK < NB:
                    dlr_issue(b + PPK)
                if b + PPK - 1 < NB:
                    prepass_block(b + PPK - 1)
                if b + 2 < NB:
                    issue_gatherB(b + 2)
                if pend is not None:
                    epilogue(*pend)
                    pend = None
                if b + 1 < NB:
                    build_masks(b + 1)

                # hp = [w * h | w]  (pair-packed 2x multiply, split in halves
                # so aggregation can start on the first half early)
                hp = sm.tile([P, T, AGC], dt.bfloat16, tag="hp")
                T1 = max(T // 2, 1)
                agg = psB.tile([P, AGC], dt.float32, tag="agg")
                for t0, t1 in ((0, T1), (T1, T)):
                    if t0 >= t1:
                        continue
                    ts = t1 - t0
                    nc.vector.tensor_tensor(
                        hp[:, t0:t1, 0:COUT].rearrange(
                            "p t (h f2 pr) -> p t h f2 pr", h=H, pr=2),
                        g[:, t0:t1, 0:COUT].rearrange(
                            "p t (h f2 pr) -> p t h f2 pr", h=H, pr=2),
                        wt2[:, t0:t1].rearrange(
                            "p t (h1 h) pr -> p t h h1 pr", h1=1
                        ).broadcast_to([P, ts, H, F // 2, 2]),
                        mybir.AluOpType.mult)
                    nc.vector.tensor_copy(
                        hp[:, t0:t1, COUT:AGC],
                        wt2[:, t0:t1, :, 0])
                    for t in range(t0, t1):
                        nc.tensor.matmul(agg[:], lhsT=mk[:, t, :],
                                         rhs=hp[:, t, :],
                                         start=(t == 0), stop=(t == T - 1))
                g_sb[b] = None
                mk_sb[b] = None
                pend = (b, agg)
            epilogue(*pend)
            if layer == 1:
                nc.sync.dma_start(
                    outT2[:].rearrange("(t n) c -> n t c", t=NB), oall[:])
            else:
                nc.sync.dma_start(
                    outT[:].rearrange("(t n) c -> n t c", t=NB), oall[:])

    nc.compile()
    return nc


# ----------------------------------------------------------------------------
# host-side weight packing
# ----------------------------------------------------------------------------

def _expand_att(att, H, F):
    out = np.zeros((H * F, H), np.float32)
    for h in range(H):
        out[h * F:(h + 1) * F, h] = att[h]
    return out


def _inputs_layer(meta, idx_all, dstl_pt, dstl_row, b, layer):
    H = H1 if layer == 1 else 1
    COUT = H * F1
    b_np = np.asarray(b, np.float32).reshape(1, COUT)
    ior = np.arange(P, dtype=np.float32).reshape(1, P).astype(BF16)
    ioc = np.arange(P, dtype=np.float32).reshape(P, 1)
    in_maps = []
    for m in range(NCORES):
        in_maps.append({
            "idxT": idx_all[m],
            "dstlT": dstl_pt[m].astype(BF16),
            "dstlR": dstl_row[m],
            "brow": b_np, "iot_r": ior, "iot_c": ioc,
        })
    return in_maps


# ----------------------------------------------------------------------------
# entry point
# ----------------------------------------------------------------------------

def kernel(x, edge_index, W1, att_src1, att_dst1, b1, W2, att_src2, att_dst2,
           b2):
    x = np.asarray(x, np.float32)
    edge_index = np.asarray(edge_index)

    meta, idx_all, dstl_pt, dstl_row = _prep_edges(edge_index)
    asg = np.asarray(meta["asg"])                     # [NB, NCORES]

    # ---- launch 0: per-node projection [h | asrc | adst] ----
    key0 = (0,)
    if key0 not in _prog_cache:
        _prog_cache[key0] = _build_proj()
    nc0 = _prog_cache[key0]

    W1f = np.asarray(W1, np.float32)
    wasrc1 = W1f @ _expand_att(np.asarray(att_src1, np.float32), H1, F1)
    wadst1 = W1f @ _expand_att(np.asarray(att_dst1, np.float32), H1, F1)
    wr_np = np.concatenate([W1f, wasrc1, wadst1], axis=1)
    wr_np = np.ascontiguousarray(wr_np.reshape(1, P, 256 + 2 * H1)).astype(BF16)

    xpad = np.zeros((NCORES * SHARD, IN), np.float32)
    xpad[:N] = x
    in_maps0 = []
    for m in range(NCORES):
        shard = xpad[m * SHARD:(m + 1) * SHARD]
        xs_np = np.ascontiguousarray(
            shard.reshape(NB, P, 1, P).transpose(2, 3, 0, 1)).astype(BF16)
        in_maps0.append({"xs": xs_np, "wr": wr_np})
    res0 = run_bass_kernel_spmd(nc0, in_maps0, list(range(NCORES))).results

    COUT1 = H1 * F1
    htab1 = np.zeros((NPAD, 384), BF16)
    adst1 = np.zeros((N + P, H1), np.float32)         # per-node adst (layer 1)
    for m in range(NCORES):
        lo = m * SHARD
        hi = min(NPAD, (m + 1) * SHARD)
        hts = res0[m]["hts"][:hi - lo]
        htab1[lo:hi, 0:COUT1 + 2 * H1] = hts[:, 0:COUT1 + 2 * H1]
        adst1[lo:hi] = hts[:, COUT1 + 2 * H1:].copy().view(np.float32)

    # per-core adst in assigned-block order: [P, NB*H] (partition = dst local)
    def adst_input(adst_n, H):
        out = []
        for m in range(NCORES):
            a = np.zeros((NB, P, H), np.float32)
            for i in range(NB):
                g = asg[i, m]
                rows = adst_n[g * P:(g + 1) * P]
                a[i, :len(rows)] = rows
            out.append(np.ascontiguousarray(
                a.transpose(1, 0, 2).reshape(P, NB * H)).astype(BF16))
        return out

    # ---- layer 1 ----
    tkey = tuple(meta["Tm"])
    zb1 = bool(np.all(np.asarray(b1) == 0))
    key1 = (1, tkey, zb1)
    if key1 not in _prog_cache:
        _prog_cache[key1] = _build_layer(meta, 1, zb1)
    ncA = _prog_cache[key1]
    in_maps = _inputs_layer(meta, idx_all, dstl_pt, dstl_row, b1, 1)
    W2f = np.asarray(W2, np.float32)
    wasrc2 = W2f @ np.asarray(att_src2, np.float32).reshape(F1, 1)
    wadst2 = W2f @ np.asarray(att_dst2, np.float32).reshape(F1, 1)
    w2r_np = np.ascontiguousarray(
        np.concatenate([W2f, wasrc2, wadst2], axis=1).reshape(2, P, 66)
    ).astype(BF16)
    ident = np.eye(P, dtype=np.float32).astype(BF16)
    a1in = adst_input(adst1, H1)
    for m, mmap in enumerate(in_maps):
        mmap["w2r"] = w2r_np
        mmap["identT"] = ident
        mmap["htab"] = htab1
        mmap["adstT"] = a1in[m]
    resA = run_bass_kernel_spmd(ncA, in_maps, list(range(NCORES))).results

    # reassemble layer-2 table + adst2 from assigned-block outputs
    htab2 = np.zeros((NPAD, 128), BF16)
    adst2 = np.zeros((N + P, 1), np.float32)
    for m in range(NCORES):
        o2 = resA[m]["outT2"]
        for i in range(NB):
            g = asg[i, m]
            if g * P >= NPAD:
                continue
            hi = min(NPAD, (g + 1) * P) - g * P
            htab2[g * P:g * P + hi, 0:66] = o2[i * P:i * P + hi, 0:66]
            adst2[g * P:g * P + hi, 0] = (
                o2[i * P:i * P + hi, 66:68].copy().view(np.float32)[:, 0])

    # ---- layer 2 ----
    zb2 = bool(np.all(np.asarray(b2) == 0))
    key2 = (2, tkey, zb2)
    if key2 not in _prog_cache:
        _prog_cache[key2] = _build_layer(meta, 2, zb2)
    ncB = _prog_cache[key2]
    in_maps2 = _inputs_layer(meta, idx_all, dstl_pt, dstl_row, b2, 2)
    a2in = adst_input(adst2, 1)
    for m, mmap in enumerate(in_maps2):
        mmap["htab"] = htab2
        mmap["adstT"] = a2in[m]
    resB = run_bass_kernel_spmd(ncB, in_maps2, list(range(NCORES))).results

    out = np.zeros((N, F1), np.float32)
    for m in range(NCORES):
        o = resB[m]["outT"]
        for i in range(NB):
            g = asg[i, m]
            lo = g * P
            if lo >= N:
                continue
            hi = min(N, lo + P)
            out[lo:hi] = o[i * P:i * P + (hi - lo)].astype(np.float32)
    return out


# revision 14
# speedup vs baseline: 1.0157x; 1.0157x over previous
"""2-layer GAT on 8 Trainium2 NeuronCores (Bass/Tile).

Sharding: the 391 dst 128-node blocks are sorted by half-A edge count and
dealt in groups of 8 to the cores (one block per core per iteration), so the
per-iteration cross-core tile maximum stays near the mean.  Edges are routed
to the core owning their dst block and laid out in shared tiles: half-A rows
(table rows < SPLIT, int16-indexable) first, padded to the iteration max,
then half-B rows in the same tile array (gather B runs first with leading
dummy indices, gather A then overwrites its region).

Per-layer device program (phase B only; projections are fused elsewhere):
  per 128-dst block: dma_gather B + A from the HBM row table
  [h | asrc f32-bits], one-hot dst masks on DVE (pair-packed 2x),
  per-edge adst via maskT matmuls (PSUM-resident), w = exp(prelu(asrc+adst))
  on ACT, weighted rows on DVE, aggregation + softmax denominator via
  PSUM-accumulated matmuls, epilogue scaling on ACT.

Launch 0 projects [h | asrc | adst] per node; layer 1's epilogue fuses the
layer-2 projection [h2 | asrc2 | adst2], so neither layer loads x at all.
Shards are exchanged through the host between launches.
"""

import os
import numpy as np
import ml_dtypes

import concourse.bass as bass
import concourse.bacc as bacc
import concourse.tile as tile
from concourse import mybir
from concourse.bass_utils import run_bass_kernel_spmd

BF16 = ml_dtypes.bfloat16

N = 50000
E = 800000
IN = 128
H1 = 4
F1 = 64
NEG = 0.2
P = 128
NCORES = 8
NB = 49                 # block iterations per core
SHARD = NB * P          # 6272 rows per core in the table
NPAD = 391 * P          # 50048 padded node count
NGB = NCORES * NB       # 392 block slots (391 real + 1 dummy)
SPLIT = 196 * P         # 25088: gather-table half boundary (int16 idx limit)
GRP = 16                # proj-launch load group

_prog_cache = {}


# ----------------------------------------------------------------------------
# host-side edge preprocessing (shared by both layers)
# ----------------------------------------------------------------------------

def _prep_edges(edge_index):
    src = np.concatenate([edge_index[0].astype(np.int64), np.arange(N, dtype=np.int64)])
    dst = np.concatenate([edge_index[1].astype(np.int64), np.arange(N, dtype=np.int64)])
    order = np.argsort(dst, kind="stable")
    s = src[order]
    d = dst[order]

    gb = d >> 7                                   # global 128-block of dst
    cnt = np.bincount(gb, minlength=NGB)
    starts = np.concatenate([[0], np.cumsum(cnt)])
    isB = s >= SPLIT
    cntA = np.zeros(NGB, np.int64)
    for g in range(NGB):
        cntA[g] = np.count_nonzero(~isB[starts[g]:starts[g + 1]])

    # deal blocks sorted by half-A count: iteration i gets ranks [8i, 8i+8)
    blk_order = np.argsort(-cntA, kind="stable")
    asg = blk_order.reshape(NB, NCORES)           # [iter, core] -> global block
    nA = cntA[asg]                                # [NB, NCORES]
    nBc = (cnt - cntA)[asg]
    nAmax = nA.max(1)                             # [NB]
    nBmax = nBc.max(1)
    Tm = np.maximum(-(-(nAmax + nBmax) // P), 1).astype(np.int64)
    niA16 = (-(-nAmax // 16) * 16).astype(np.int64)   # static gather-A num_idxs
    fA = nAmax // P                               # full A tiles
    rA = nAmax % P                                # B's leading dummy count
    niB = (Tm - fA) * P                           # static gather-B num_idxs

    toff = np.zeros(NB + 1, np.int64)
    np.cumsum(Tm, out=toff[1:])
    Ttot = int(toff[NB])
    TMX = int(Tm.max())

    scol = np.zeros(NB + 1, np.int64)             # idx column offsets (per 16)
    np.cumsum(niA16 // 16 + niB // 16, out=scol[1:])
    Stot = int(scol[NB])

    idx_all = np.zeros((NCORES, P, Stot), np.int16)
    dstl = np.full((NCORES, Ttot, P), -1.0, np.float32)   # [t, p] layout

    for i in range(NB):
        sA = int(scol[i])
        sB = sA + int(niA16[i] // 16)
        for m in range(NCORES):
            g = asg[i, m]
            e0, e1 = starts[g], starts[g + 1]
            sb = s[e0:e1]
            mB = isB[e0:e1]
            shA = sb[~mB]
            shB = sb[mB] - SPLIT
            dlA = (d[e0:e1][~mB] - (g << 7)).astype(np.float32)
            dlB = (d[e0:e1][mB] - (g << 7)).astype(np.float32)
            na, nb_ = len(shA), len(shB)
            # gather-A idx: real | dummy-0 to nAmax | -1 tail to niA16
            ia = np.zeros(int(niA16[i]), np.int16)
            ia[:na] = shA
            ia[int(nAmax[i]):] = -1
            # gather-B idx: rA dummy-0 | real | dummy-0 tail
            ib = np.zeros(int(niB[i]), np.int16)
            ib[int(rA[i]):int(rA[i]) + nb_] = shB
            for seg, off in ((ia, sA), (ib, sB)):
                w = seg.reshape(-1, 16).T          # [16, S]
                idx_all[m][:, off:off + w.shape[1]] = np.tile(w, (8, 1))
            dl = np.full(int(Tm[i]) * P, -1.0, np.float32)
            dl[:na] = dlA
            dl[int(nAmax[i]):int(nAmax[i]) + nb_] = dlB
            dstl[m][toff[i]:toff[i] + Tm[i], :] = dl.reshape(int(Tm[i]), P)

    dstl_pt = np.ascontiguousarray(dstl.transpose(0, 2, 1))   # [m, P, Ttot]
    dstl_row = np.full((NCORES, NB, TMX * P), -1.0, np.float32)
    for m in range(NCORES):
        for i in range(NB):
            T = int(Tm[i])
            dstl_row[m, i, :T * P] = dstl[m, toff[i]:toff[i] + T].reshape(-1)
    dstl_row = dstl_row.astype(BF16)

    meta = dict(Tm=Tm.tolist(), toff=toff.tolist(), fA=fA.tolist(),
                rA=rA.tolist(), niA16=niA16.tolist(), niB=niB.tolist(),
                scol=scol.tolist(), Ttot=Ttot, Stot=Stot, Tmax=TMX,
                asg=asg.tolist())
    return meta, idx_all, dstl_pt, dstl_row


# ----------------------------------------------------------------------------
# launch 0: project own shard -> [h | asrc | adst] table slice
# ----------------------------------------------------------------------------

def _build_proj():
    dt = mybir.dt
    KCH, H = 1, H1
    COUT = H1 * F1
    RC = COUT + 2 * H                   # [h | asrc | adst]
    OCOL = COUT + 4 * H                 # bf16 slots: h | asrc bits | adst bits
    nc = bacc.Bacc("TRN2", target_bir_lowering=False, debug=False,
                   num_devices=NCORES)
    xs = nc.dram_tensor("xs", [KCH, P, NB, P], dt.bfloat16,
                        kind="ExternalInput")
    wr = nc.dram_tensor("wr", [KCH, P, RC], dt.bfloat16,
                        kind="ExternalInput")
    hts = nc.dram_tensor("hts", [SHARD, OCOL], dt.bfloat16,
                         kind="ExternalOutput")
    with tile.TileContext(nc) as tc:
        with (
            tc.tile_pool(name="const", bufs=1) as cp,
            tc.tile_pool(name="pa", bufs=3) as pa,
            tc.tile_pool(name="psA", bufs=3, space="PSUM") as psA,
        ):
            wr_sb = cp.tile([P, KCH, RC], dt.bfloat16)
            nc.sync.dma_start(wr_sb[:], wr[:].rearrange("k p c -> p k c"))
            for g0 in range(0, NB, GRP):
                gn = min(GRP, NB - g0)
                xa = pa.tile([P, KCH, gn, P], dt.bfloat16, tag="xa")
                nc.sync.dma_start(
                    xa[:], xs[:, :, g0:g0 + gn, :].rearrange(
                        "k f t n -> f k t n"))
                hst = pa.tile([P, gn, OCOL], dt.bfloat16, tag="hst")
                for t0 in range(0, gn, 2):
                    pn = min(2, gn - t0)
                    ps = psA.tile([P, 2, 512], dt.float32, tag="psa")
                    for t2 in range(pn):
                        for k in range(KCH):
                            nc.tensor.matmul(ps[:, t2, 0:RC],
                                             lhsT=xa[:, k, t0 + t2, :],
                                             rhs=wr_sb[:, k, 0:RC],
                                             start=(k == 0),
                                             stop=(k == KCH - 1))
                    nc.scalar.activation(
                        hst[:, t0:t0 + pn, 0:COUT], ps[:, 0:pn, 0:COUT],
                        mybir.ActivationFunctionType.Copy)
                    nc.vector.tensor_copy(
                        hst[:, t0:t0 + pn, COUT:OCOL].bitcast(dt.float32),
                        ps[:, 0:pn, COUT:COUT + 2 * H])
                nc.sync.dma_start(
                    hts[g0 * P:(g0 + gn) * P, :].rearrange(
                        "(t n) c -> n t c", t=gn),
                    hst[:])
    nc.compile()
    return nc


# ----------------------------------------------------------------------------
# per-layer message-passing program (phase B)
# ----------------------------------------------------------------------------

def _build_layer(meta, layer, zero_bias):
    """layer 1: heads 4, F 64, fused layer-2 row production, no dense out.
    layer 2: heads 1, F 64, out f32 [SHARD, 64]."""
    dt = mybir.dt
    Tm, toff, fAm = meta["Tm"], meta["toff"], meta["fA"]
    niA16, niB, scol = meta["niA16"], meta["niB"], meta["scol"]
    Ttot, Stot, Tmax = meta["Ttot"], meta["Stot"], meta["Tmax"]

    if layer == 1:
        H, F = H1, F1
    else:
        H, F = 1, F1
    COUT = H * F
    AGC = COUT + H                    # aggregation psum cols: [num | den]
    TABC = 384 if layer == 1 else 128  # table row slots (256B granules)

    nc = bacc.Bacc("TRN2", target_bir_lowering=False, debug=False,
                   num_devices=NCORES)

    if layer == 1:
        w2r = nc.dram_tensor("w2r", [2, P, 66], dt.bfloat16,
                             kind="ExternalInput")
        identT = nc.dram_tensor("identT", [P, P], dt.bfloat16,
                                kind="ExternalInput")
        outT2 = nc.dram_tensor("outT2", [SHARD, 68], dt.bfloat16,
                               kind="ExternalOutput")
    else:
        outT = nc.dram_tensor("outT", [SHARD, COUT], dt.float32,
                              kind="ExternalOutput")
    idxT = nc.dram_tensor("idxT", [P, Stot], dt.int16, kind="ExternalInput")
    dstlT = nc.dram_tensor("dstlT", [P, Ttot], dt.bfloat16, kind="ExternalInput")
    dstlR = nc.dram_tensor("dstlR", [NB, Tmax * P], dt.bfloat16,
                           kind="ExternalInput")
    adstT = nc.dram_tensor("adstT", [P, NB * H], dt.bfloat16,
                           kind="ExternalInput")
    brow = nc.dram_tensor("brow", [1, COUT], dt.float32, kind="ExternalInput")
    iot_r = nc.dram_tensor("iot_r", [1, P], dt.bfloat16, kind="ExternalInput")
    iot_c = nc.dram_tensor("iot_c", [P, 1], dt.float32, kind="ExternalInput")
    htab = nc.dram_tensor("htab", [NPAD, TABC], dt.bfloat16,
                          kind="ExternalInput")

    SP = bool(int(os.environ.get("GAT_SP", "0")))
    PBB = int(os.environ.get("GAT_PBB", "4"))
    PPK = int(os.environ.get("GAT_PPK", "3"))
    PBM = int(os.environ.get("GAT_PB_MOD", "2"))

    with tile.TileContext(nc) as tc:
        with (
            tc.tile_pool(name="const", bufs=1) as cp,
            tc.tile_pool(name="keep", bufs=1) as kp,
            tc.tile_pool(name="pp", bufs=3) as ppool,
            tc.tile_pool(name="pb", bufs=PBB) as pb,
            tc.tile_pool(name="sm", bufs=3) as sm,
            tc.tile_pool(name="psA", bufs=1, space="PSUM") as psA,
            tc.tile_pool(name="psB", bufs=2, space="PSUM") as psB,
            tc.tile_pool(name="psD", bufs=PPK + 1, space="PSUM") as psD,
        ):
            # ---- resident constants ----
            b_sb = cp.tile([P, COUT], dt.float32)
            nc.sync.dma_start(b_sb[:], brow[:].broadcast_to([P, COUT]))
            ior_sb = cp.tile([P, P], dt.bfloat16)
            nc.sync.dma_start(ior_sb[:], iot_r[:].broadcast_to([P, P]))
            ioc_sb = cp.tile([P, 1], dt.float32)
            nc.sync.dma_start(ioc_sb[:], iot_c[:])
            if layer == 1:
                w2_sb = cp.tile([P, 2, 66], dt.bfloat16)
                nc.sync.dma_start(w2_sb[:], w2r[:].rearrange("k p c -> p k c"))
                id_sb = cp.tile([P, P], dt.bfloat16)
                nc.sync.dma_start(id_sb[:], identT[:])
            idx_sb = kp.tile([P, Stot], dt.int16)
            nc.sync.dma_start(idx_sb[:], idxT[:])
            dstl_sb = kp.tile([P, Ttot], dt.bfloat16)
            nc.sync.dma_start(dstl_sb[:], dstlT[:])
            adst_sh = kp.tile([P, NB * H], dt.bfloat16)
            nc.sync.dma_start(adst_sh[:], adstT[:])

            # ---- pre-pass: expand adst to per-edge values (PSUM-resident),
            # staged so the dlr broadcast has a full iteration to land ----
            adst_ps = [None] * NB
            dlr_sb = [None] * NB

            def dlr_issue(b):
                T = Tm[b]
                dlr = ppool.tile([P, T * P], dt.bfloat16, tag="dlr")
                if PBM and b % PBM:
                    dlrow = ppool.tile([1, T * P], dt.bfloat16, tag="dlrow")
                    nc.sync.dma_start(dlrow[:], dstlR[b:b + 1, 0:T * P])
                    nc.gpsimd.partition_broadcast(dlr[:], dlrow[:])
                else:
                    nc.sync.dma_start(
                        dlr[:],
                        dstlR[b:b + 1, 0:T * P].broadcast_to([P, T * P]))
                dlr_sb[b] = dlr

            def prepass_block(b):
                T = Tm[b]
                mT = ppool.tile([P, T, P], dt.bfloat16, tag="mT")
                nc.vector.tensor_scalar(
                    mT[:].rearrange("p t e -> p (t e)"), dlr_sb[b][:],
                    ioc_sb[:], None, mybir.AluOpType.is_equal)
                dlr_sb[b] = None
                ap_ps = psD.tile([P, T * H], dt.float32, tag="adps")
                for t in range(T):
                    nc.tensor.matmul(ap_ps[:, t * H:(t + 1) * H],
                                     lhsT=mT[:, t, :],
                                     rhs=adst_sh[:, b * H:(b + 1) * H],
                                     start=True, stop=True)
                adst_ps[b] = ap_ps

            # ---- gathers: B first (covers tail incl. boundary dummies),
            # then A overwrites its region ----
            htabA = htab[0:SPLIT, :]
            htabB = htab[SPLIT:NPAD, :]

            g_sb = [None] * NB
            mk_sb = [None] * NB

            def issue_gatherB(b):
                T = Tm[b]
                g = pb.tile([P, T, TABC], dt.bfloat16, tag="gath")
                sB = scol[b] + niA16[b] // 16
                if niB[b] > 0:
                    nc.gpsimd.dma_gather(
                        g[:, fAm[b]:T, :], htabB,
                        idx_sb[:, sB:sB + niB[b] // 16],
                        niB[b], niB[b], TABC, single_packet=SP)
                g_sb[b] = g

            def issue_gatherA(b):
                T = Tm[b]
                g = g_sb[b]
                sA = scol[b]
                if niA16[b] > 0:
                    a_tiles = -(-niA16[b] // P)
                    nc.gpsimd.dma_gather(
                        g[:, 0:a_tiles, :], htabA,
                        idx_sb[:, sA:sA + niA16[b] // 16],
                        niA16[b], niA16[b], TABC, single_packet=SP)

            def build_masks(b):
                T = Tm[b]
                # dst one-hot masks (pair-packed for DVE 2x)
                dl2 = sm.tile([P, T, 2], dt.bfloat16, tag="dl2")
                nc.vector.tensor_copy(
                    dl2[:],
                    dstl_sb[:, toff[b]:toff[b] + T].rearrange(
                        "p (t o) -> p t o", o=1).broadcast_to([P, T, 2]))
                mk = sm.tile([P, T, P], dt.bfloat16, tag="mk")   # [e_p,(t,d)]
                nc.vector.tensor_tensor(
                    mk[:].rearrange("p t (d2 pr) -> p t d2 pr", pr=2),
                    ior_sb[:].rearrange("p (t d2 pr) -> p t d2 pr", t=1, pr=2
                                        ).broadcast_to([P, T, P // 2, 2]),
                    dl2[:].rearrange("p t (d2 pr) -> p t d2 pr", d2=1
                                     ).broadcast_to([P, T, P // 2, 2]),
                    mybir.AluOpType.is_equal)
                mk_sb[b] = mk

            def epilogue(b, agg):
                # out = num/(den+eps) (+bias) (+ELU and fused proj, layer 1)
                dn = sm.tile([P, H], dt.float32, tag="dn")
                nc.vector.tensor_scalar_add(dn[:], agg[:, COUT:AGC], 1e-16)
                rc = sm.tile([P, H], dt.float32, tag="rc")
                nc.vector.reciprocal(rc[:], dn[:])
                if layer == 1:
                    ob = sm.tile([P, COUT], dt.bfloat16, tag="ob")
                    for h in range(H):
                        nc.scalar.activation(ob[:, h * F:(h + 1) * F],
                                             agg[:, h * F:(h + 1) * F],
                                             mybir.ActivationFunctionType.Copy,
                                             scale=rc[:, h:h + 1])
                    if not zero_bias:
                        nc.vector.tensor_add(
                            ob[:], ob[:],
                            b_sb[:].bitcast(dt.bfloat16)[:, 1::2])
                    # elu(y) = relu(y) + exp(min(y,0)) - 1
                    r1 = sm.tile([P, COUT], dt.bfloat16, tag="r1")
                    nc.scalar.activation(r1[:], ob[:],
                                         mybir.ActivationFunctionType.Relu,
                                         scale=-1.0)
                    r2 = sm.tile([P, COUT], dt.bfloat16, tag="r2")
                    nc.scalar.activation(r2[:], r1[:],
                                         mybir.ActivationFunctionType.Exp,
                                         scale=-1.0)
                    nc.scalar.activation(ob[:], ob[:],
                                         mybir.ActivationFunctionType.Relu)
                    nc.vector.scalar_tensor_tensor(
                        ob[:], r2[:], -1.0, ob[:],
                        mybir.AluOpType.add, mybir.AluOpType.add)
                    # fused layer-2 row production:
                    # [elu(out1) @ [W2|wasrc2|wadst2]] -> [h2|asrc2|adst2]
                    ps_t = psA.tile([P, 2, P], dt.bfloat16, tag="pst")
                    for c in range(2):
                        nc.tensor.transpose(ps_t[:, c, :],
                                            ob[:, c * P:(c + 1) * P],
                                            id_sb[:])
                    x2T = sm.tile([P, 2, P], dt.bfloat16, tag="x2T")
                    nc.scalar.activation(x2T[:], ps_t[:],
                                         mybir.ActivationFunctionType.Copy)
                    ps2 = psA.tile([P, 66], dt.float32, tag="ps2")
                    for c in range(2):
                        nc.tensor.matmul(ps2[:], lhsT=x2T[:, c, :],
                                         rhs=w2_sb[:, c, :],
                                         start=(c == 0), stop=(c == 1))
                    hst2 = sm.tile([P, 68], dt.bfloat16, tag="hst2")
                    nc.scalar.activation(hst2[:, 0:64], ps2[:, 0:64],
                                         mybir.ActivationFunctionType.Copy)
                    nc.vector.tensor_copy(
                        hst2[:, 64:68].bitcast(dt.float32), ps2[:, 64:66])
                    nc.sync.dma_start(outT2[b * P:(b + 1) * P, :], hst2[:])
                else:
                    ob = sm.tile([P, COUT], dt.float32, tag="ob")
                    nc.scalar.activation(ob[:], agg[:, 0:COUT],
                                         mybir.ActivationFunctionType.Copy,
                                         scale=rc[:, 0:1])
                    if not zero_bias:
                        nc.vector.tensor_add(ob[:], ob[:], b_sb[:])
                    nc.sync.dma_start(outT[b * P:(b + 1) * P, :], ob[:])

            # ---- phase B: software-pipelined per-block message passing.
            # Emission order is tuned for the in-order engine queues: the
            # et->prelu->exp->hp critical chain leads, lookahead issues fill
            # the ACT round-trip, the lagged epilogue never blocks it. ----
            for q in range(min(PPK, NB)):
                dlr_issue(q)
            for q in range(min(PPK - 1, NB)):
                prepass_block(q)
            for q in range(min(2, NB)):
                issue_gatherB(q)
            issue_gatherA(0)
            build_masks(0)
            pend = None                     # (block, agg) awaiting epilogue
            for b in range(NB):
                T = Tm[b]
                g = g_sb[b]
                mk = mk_sb[b]

                # w2 = exp(prelu(asrc + adst)) pair-broadcast, on ACT
                et = sm.tile([P, T * H], dt.float32, tag="et")
                nc.vector.tensor_tensor(
                    et[:].rearrange("p (t h) -> p t h", h=H),
                    g[:, :, COUT:COUT + 2 * H].bitcast(dt.float32),
                    adst_ps[b][:].rearrange("p (t h) -> p t h", h=H),
                    mybir.AluOpType.add)
                adst_ps[b] = None
                lr = sm.tile([P, T * H], dt.float32, tag="lr")
                nc.scalar.activation(lr[:], et[:],
                                     mybir.ActivationFunctionType.Prelu,
                                     alpha=NEG)
                wt2 = sm.tile([P, T, H, 2], dt.bfloat16, tag="wt2")
                nc.scalar.activation(
                    wt2[:],
                    lr[:].rearrange("p (t h o) -> p t h o", h=H, o=1
                                    ).broadcast_to([P, T, H, 2]),
                    mybir.ActivationFunctionType.Exp)

                # lookahead issues (fill the ACT round-trip gap on DVE/Pool;
                # the critical gather A goes first in the DMA queue)
                if b + 1 < NB:
                    issue_gatherA(b + 1)
                if b + PPK < NB:
                    dlr_issue(b + PPK)
                if b + PPK - 1 < NB:
                    prepass_block(b + PPK - 1)
                if b + 2 < NB:
                    issue_gatherB(b + 2)
                if pend is not None:
                    epilogue(*pend)
                    pend = None
                if b + 1 < NB:
                    build_masks(b + 1)

                # hp = [w * h | w]  (pair-packed 2x multiply, split in halves
                # so aggregation can start on the first half early)
                hp = sm.tile([P, T, AGC], dt.bfloat16, tag="hp")
                T1 = max(T // 2, 1)
                agg = psB.tile([P, AGC], dt.float32, tag="agg")
                for t0, t1 in ((0, T1), (T1, T)):
                    if t0 >= t1:
                        continue
                    ts = t1 - t0
                    nc.vector.tensor_tensor(
                        hp[:, t0:t1, 0:COUT].rearrange(
                            "p t (h f2 pr) -> p t h f2 pr", h=H, pr=2),
                        g[:, t0:t1, 0:COUT].rearrange(
                            "p t (h f2 pr) -> p t h f2 pr", h=H, pr=2),
                        wt2[:, t0:t1].rearrange(
                            "p t (h1 h) pr -> p t h h1 pr", h1=1
                        ).broadcast_to([P, ts, H, F // 2, 2]),
                        mybir.AluOpType.mult)
                    nc.vector.tensor_copy(
                        hp[:, t0:t1, COUT:AGC],
                        wt2[:, t0:t1, :, 0])
                    for t in range(t0, t1):
                        nc.tensor.matmul(agg[:], lhsT=mk[:, t, :],
                                         rhs=hp[:, t, :],
                                         start=(t == 0), stop=(t == T - 1))
                g_sb[b] = None
                mk_sb[b] = None
                pend = (b, agg)
            epilogue(*pend)

    nc.compile()
    return nc


# ----------------------------------------------------------------------------
# host-side weight packing
# ----------------------------------------------------------------------------

def _expand_att(att, H, F):
    out = np.zeros((H * F, H), np.float32)
    for h in range(H):
        out[h * F:(h + 1) * F, h] = att[h]
    return out


def _inputs_layer(meta, idx_all, dstl_pt, dstl_row, b, layer):
    H = H1 if layer == 1 else 1
    COUT = H * F1
    b_np = np.asarray(b, np.float32).reshape(1, COUT)
    ior = np.arange(P, dtype=np.float32).reshape(1, P).astype(BF16)
    ioc = np.arange(P, dtype=np.float32).reshape(P, 1)
    in_maps = []
    for m in range(NCORES):
        in_maps.append({
            "idxT": idx_all[m],
            "dstlT": dstl_pt[m].astype(BF16),
            "dstlR": dstl_row[m],
            "brow": b_np, "iot_r": ior, "iot_c": ioc,
        })
    return in_maps


# ----------------------------------------------------------------------------
# entry point
# ----------------------------------------------------------------------------

def kernel(x, edge_index, W1, att_src1, att_dst1, b1, W2, att_src2, att_dst2,
           b2):
    x = np.asarray(x, np.float32)
    edge_index = np.asarray(edge_index)

    meta, idx_all, dstl_pt, dstl_row = _prep_edges(edge_index)
    asg = np.asarray(meta["asg"])                     # [NB, NCORES]

    # ---- launch 0: per-node projection [h | asrc | adst] ----
    key0 = (0,)
    if key0 not in _prog_cache:
        _prog_cache[key0] = _build_proj()
    nc0 = _prog_cache[key0]

    W1f = np.asarray(W1, np.float32)
    wasrc1 = W1f @ _expand_att(np.asarray(att_src1, np.float32), H1, F1)
    wadst1 = W1f @ _expand_att(np.asarray(att_dst1, np.float32), H1, F1)
    wr_np = np.concatenate([W1f, wasrc1, wadst1], axis=1)
    wr_np = np.ascontiguousarray(wr_np.reshape(1, P, 256 + 2 * H1)).astype(BF16)

    xpad = np.zeros((NCORES * SHARD, IN), np.float32)
    xpad[:N] = x
    in_maps0 = []
    for m in range(NCORES):
        shard = xpad[m * SHARD:(m + 1) * SHARD]
        xs_np = np.ascontiguousarray(
            shard.reshape(NB, P, 1, P).transpose(2, 3, 0, 1)).astype(BF16)
        in_maps0.append({"xs": xs_np, "wr": wr_np})
    res0 = run_bass_kernel_spmd(nc0, in_maps0, list(range(NCORES))).results

    COUT1 = H1 * F1
    htab1 = np.zeros((NPAD, 384), BF16)
    adst1 = np.zeros((N + P, H1), np.float32)         # per-node adst (layer 1)
    for m in range(NCORES):
        lo = m * SHARD
        hi = min(NPAD, (m + 1) * SHARD)
        hts = res0[m]["hts"][:hi - lo]
        htab1[lo:hi, 0:COUT1 + 2 * H1] = hts[:, 0:COUT1 + 2 * H1]
        adst1[lo:hi] = hts[:, COUT1 + 2 * H1:].copy().view(np.float32)

    # per-core adst in assigned-block order: [P, NB*H] (partition = dst local)
    def adst_input(adst_n, H):
        out = []
        for m in range(NCORES):
            a = np.zeros((NB, P, H), np.float32)
            for i in range(NB):
                g = asg[i, m]
                rows = adst_n[g * P:(g + 1) * P]
                a[i, :len(rows)] = rows
            out.append(np.ascontiguousarray(
                a.transpose(1, 0, 2).reshape(P, NB * H)).astype(BF16))
        return out

    # ---- layer 1 ----
    tkey = tuple(meta["Tm"])
    zb1 = bool(np.all(np.asarray(b1) == 0))
    key1 = (1, tkey, zb1)
    if key1 not in _prog_cache:
        _prog_cache[key1] = _build_layer(meta, 1, zb1)
    ncA = _prog_cache[key1]
    in_maps = _inputs_layer(meta, idx_all, dstl_pt, dstl_row, b1, 1)
    W2f = np.asarray(W2, np.float32)
    wasrc2 = W2f @ np.asarray(att_src2, np.float32).reshape(F1, 1)
    wadst2 = W2f @ np.asarray(att_dst2, np.float32).reshape(F1, 1)
    w2r_np = np.ascontiguousarray(
        np.concatenate([W2f, wasrc2, wadst2], axis=1).reshape(2, P, 66)
    ).astype(BF16)
    ident = np.eye(P, dtype=np.float32).astype(BF16)
    a1in = adst_input(adst1, H1)
    for m, mmap in enumerate(in_maps):
        mmap["w2r"] = w2r_np
        mmap["identT"] = ident
        mmap["htab"] = htab1
        mmap["adstT"] = a1in[m]
    resA = run_bass_kernel_spmd(ncA, in_maps, list(range(NCORES))).results

    # reassemble layer-2 table + adst2 from assigned-block outputs
    htab2 = np.zeros((NPAD, 128), BF16)
    adst2 = np.zeros((N + P, 1), np.float32)
    for m in range(NCORES):
        o2 = resA[m]["outT2"]
        for i in range(NB):
            g = asg[i, m]
            if g * P >= NPAD:
                continue
            hi = min(NPAD, (g + 1) * P) - g * P
            htab2[g * P:g * P + hi, 0:66] = o2[i * P:i * P + hi, 0:66]
            adst2[g * P:g * P + hi, 0] = (
                o2[i * P:i * P + hi, 66:68].copy().view(np.float32)[:, 0])

    # ---- layer 2 ----
    zb2 = bool(np.all(np.asarray(b2) == 0))
    key2 = (2, tkey, zb2)
    if key2 not in _prog_cache:
        _prog_cache[key2] = _build_layer(meta, 2, zb2)
    ncB = _prog_cache[key2]
    in_maps2 = _inputs_layer(meta, idx_all, dstl_pt, dstl_row, b2, 2)
    a2in = adst_input(adst2, 1)
    for m, mmap in enumerate(in_maps2):
        mmap["htab"] = htab2
        mmap["adstT"] = a2in[m]
    resB = run_bass_kernel_spmd(ncB, in_maps2, list(range(NCORES))).results

    out = np.zeros((N, F1), np.float32)
    for m in range(NCORES):
        o = resB[m]["outT"]
        for i in range(NB):
            g = asg[i, m]
            lo = g * P
            if lo >= N:
                continue
            hi = min(N, lo + P)
            out[lo:hi] = o[i * P:i * P + (hi - lo)].astype(np.float32)
    return out


# revision 16
# speedup vs baseline: 1.0819x; 1.0652x over previous
"""2-layer GAT on 8 Trainium2 NeuronCores (Bass/Tile).

Sharding: the 391 dst 128-node blocks are sorted by half-A edge count and
dealt in groups of 8 to the cores (one block per core per iteration), so the
per-iteration cross-core tile maximum stays near the mean.  Edges are routed
to the core owning their dst block and laid out in shared tiles: half-A rows
(table rows < SPLIT, int16-indexable) first, padded to the iteration max,
then half-B rows in the same tile array (gather B runs first with leading
dummy indices, gather A then overwrites its region).

Per-layer device program (phase B only; projections are fused elsewhere):
  per 128-dst block: dma_gather B + A from the HBM row table
  [h | asrc f32-bits], one-hot dst masks on DVE (pair-packed 2x),
  per-edge adst via maskT matmuls (PSUM-resident), w = exp(prelu(asrc+adst))
  on ACT, weighted rows on DVE, aggregation + softmax denominator via
  PSUM-accumulated matmuls, epilogue scaling on ACT.

Launch 0 projects [h | asrc | adst] per node; layer 1's epilogue fuses the
layer-2 projection [h2 | asrc2 | adst2], so neither layer loads x at all.
Shards are exchanged through the host between launches.
"""

import os
import numpy as np
import ml_dtypes

import concourse.bass as bass
import concourse.bacc as bacc
import concourse.tile as tile
from concourse import mybir
from concourse.bass_utils import run_bass_kernel_spmd

BF16 = ml_dtypes.bfloat16

N = 50000
E = 800000
IN = 128
H1 = 4
F1 = 64
NEG = 0.2
P = 128
NCORES = 8
NB = 49                 # block iterations per core
SHARD = NB * P          # 6272 rows per core in the table
NPAD = 391 * P          # 50048 padded node count
NGB = NCORES * NB       # 392 block slots (391 real + 1 dummy)
SPLIT = 196 * P         # 25088: gather-table half boundary (int16 idx limit)
GRP = 16                # proj-launch load group

_prog_cache = {}


# ----------------------------------------------------------------------------
# host-side edge preprocessing (shared by both layers)
# ----------------------------------------------------------------------------

def _prep_edges(edge_index):
    src = np.concatenate([edge_index[0].astype(np.int64), np.arange(N, dtype=np.int64)])
    dst = np.concatenate([edge_index[1].astype(np.int64), np.arange(N, dtype=np.int64)])
    order = np.argsort(dst, kind="stable")
    s = src[order]
    d = dst[order]

    gb = d >> 7                                   # global 128-block of dst
    cnt = np.bincount(gb, minlength=NGB)
    starts = np.concatenate([[0], np.cumsum(cnt)])
    isB = s >= SPLIT
    cntA = np.zeros(NGB, np.int64)
    for g in range(NGB):
        cntA[g] = np.count_nonzero(~isB[starts[g]:starts[g + 1]])

    # deal blocks sorted by half-A count: iteration i gets ranks [8i, 8i+8)
    blk_order = np.argsort(-cntA, kind="stable")
    asg = blk_order.reshape(NB, NCORES)           # [iter, core] -> global block
    nA = cntA[asg]                                # [NB, NCORES]
    nBc = (cnt - cntA)[asg]
    nAmax = nA.max(1)                             # [NB]
    nBmax = nBc.max(1)
    Tm = np.maximum(-(-(nAmax + nBmax) // P), 1).astype(np.int64)
    niA16 = (-(-nAmax // 16) * 16).astype(np.int64)   # static gather-A num_idxs
    fA = nAmax // P                               # full A tiles
    rA = nAmax % P                                # B's leading dummy count
    niB = (Tm - fA) * P                           # static gather-B num_idxs

    toff = np.zeros(NB + 1, np.int64)
    np.cumsum(Tm, out=toff[1:])
    Ttot = int(toff[NB])
    TMX = int(Tm.max())

    scol = np.zeros(NB + 1, np.int64)             # idx column offsets (per 16)
    np.cumsum(niA16 // 16 + niB // 16, out=scol[1:])
    Stot = int(scol[NB])

    idx_all = np.zeros((NCORES, P, Stot), np.int16)
    dstl = np.full((NCORES, Ttot, P), -1.0, np.float32)   # [t, p] layout

    for i in range(NB):
        sA = int(scol[i])
        sB = sA + int(niA16[i] // 16)
        for m in range(NCORES):
            g = asg[i, m]
            e0, e1 = starts[g], starts[g + 1]
            sb = s[e0:e1]
            mB = isB[e0:e1]
            shA = sb[~mB]
            shB = sb[mB] - SPLIT
            dlA = (d[e0:e1][~mB] - (g << 7)).astype(np.float32)
            dlB = (d[e0:e1][mB] - (g << 7)).astype(np.float32)
            na, nb_ = len(shA), len(shB)
            # gather-A idx: real | dummy-0 to nAmax | -1 tail to niA16
            ia = np.zeros(int(niA16[i]), np.int16)
            ia[:na] = shA
            ia[int(nAmax[i]):] = -1
            # gather-B idx: rA dummy-0 | real | dummy-0 tail
            ib = np.zeros(int(niB[i]), np.int16)
            ib[int(rA[i]):int(rA[i]) + nb_] = shB
            for seg, off in ((ia, sA), (ib, sB)):
                w = seg.reshape(-1, 16).T          # [16, S]
                idx_all[m][:, off:off + w.shape[1]] = np.tile(w, (8, 1))
            dl = np.full(int(Tm[i]) * P, -1.0, np.float32)
            dl[:na] = dlA
            dl[int(nAmax[i]):int(nAmax[i]) + nb_] = dlB
            dstl[m][toff[i]:toff[i] + Tm[i], :] = dl.reshape(int(Tm[i]), P)

    dstl_pt = np.ascontiguousarray(dstl.transpose(0, 2, 1))   # [m, P, Ttot]
    dstl_row = np.full((NCORES, NB, TMX * P), -1.0, np.float32)
    for m in range(NCORES):
        for i in range(NB):
            T = int(Tm[i])
            dstl_row[m, i, :T * P] = dstl[m, toff[i]:toff[i] + T].reshape(-1)
    dstl_row = dstl_row.astype(BF16)

    meta = dict(Tm=Tm.tolist(), toff=toff.tolist(), fA=fA.tolist(),
                rA=rA.tolist(), niA16=niA16.tolist(), niB=niB.tolist(),
                scol=scol.tolist(), Ttot=Ttot, Stot=Stot, Tmax=TMX,
                asg=asg.tolist())
    return meta, idx_all, dstl_pt, dstl_row


# ----------------------------------------------------------------------------
# launch 0: project own shard -> [h | asrc | adst] table slice
# ----------------------------------------------------------------------------

def _build_proj():
    dt = mybir.dt
    KCH, H = 1, H1
    COUT = H1 * F1
    RC = COUT + 2 * H                   # [h | asrc | adst]
    OCOL = COUT + 4 * H                 # bf16 slots: h | asrc bits | adst bits
    nc = bacc.Bacc("TRN2", target_bir_lowering=False, debug=False,
                   num_devices=NCORES)
    xs = nc.dram_tensor("xs", [KCH, P, NB, P], dt.bfloat16,
                        kind="ExternalInput")
    wr = nc.dram_tensor("wr", [KCH, P, RC], dt.bfloat16,
                        kind="ExternalInput")
    hts = nc.dram_tensor("hts", [SHARD, OCOL], dt.bfloat16,
                         kind="ExternalOutput")
    with tile.TileContext(nc) as tc:
        with (
            tc.tile_pool(name="const", bufs=1) as cp,
            tc.tile_pool(name="pa", bufs=3) as pa,
            tc.tile_pool(name="psA", bufs=3, space="PSUM") as psA,
        ):
            wr_sb = cp.tile([P, KCH, RC], dt.bfloat16)
            nc.sync.dma_start(wr_sb[:], wr[:].rearrange("k p c -> p k c"))
            for g0 in range(0, NB, GRP):
                gn = min(GRP, NB - g0)
                xa = pa.tile([P, KCH, gn, P], dt.bfloat16, tag="xa")
                nc.sync.dma_start(
                    xa[:], xs[:, :, g0:g0 + gn, :].rearrange(
                        "k f t n -> f k t n"))
                hst = pa.tile([P, gn, OCOL], dt.bfloat16, tag="hst")
                for t0 in range(0, gn, 2):
                    pn = min(2, gn - t0)
                    ps = psA.tile([P, 2, 512], dt.float32, tag="psa")
                    for t2 in range(pn):
                        for k in range(KCH):
                            nc.tensor.matmul(ps[:, t2, 0:RC],
                                             lhsT=xa[:, k, t0 + t2, :],
                                             rhs=wr_sb[:, k, 0:RC],
                                             start=(k == 0),
                                             stop=(k == KCH - 1))
                    nc.scalar.activation(
                        hst[:, t0:t0 + pn, 0:COUT], ps[:, 0:pn, 0:COUT],
                        mybir.ActivationFunctionType.Copy)
                    nc.vector.tensor_copy(
                        hst[:, t0:t0 + pn, COUT:OCOL].bitcast(dt.float32),
                        ps[:, 0:pn, COUT:COUT + 2 * H])
                nc.sync.dma_start(
                    hts[g0 * P:(g0 + gn) * P, :].rearrange(
                        "(t n) c -> n t c", t=gn),
                    hst[:])
    nc.compile()
    return nc


# ----------------------------------------------------------------------------
# per-layer message-passing program (phase B)
# ----------------------------------------------------------------------------

def _build_layer(meta, layer, zero_bias):
    """layer 1: heads 4, F 64, fused layer-2 row production, no dense out.
    layer 2: heads 1, F 64, out f32 [SHARD, 64]."""
    dt = mybir.dt
    Tm, toff, fAm = meta["Tm"], meta["toff"], meta["fA"]
    niA16, niB, scol = meta["niA16"], meta["niB"], meta["scol"]
    Ttot, Stot, Tmax = meta["Ttot"], meta["Stot"], meta["Tmax"]

    if layer == 1:
        H, F = H1, F1
    else:
        H, F = 1, F1
    COUT = H * F
    AGC = COUT + H                    # aggregation psum cols: [num | den]
    TABC = 384 if layer == 1 else 128  # table row slots (256B granules)

    nc = bacc.Bacc("TRN2", target_bir_lowering=False, debug=False,
                   num_devices=NCORES)

    if layer == 1:
        w2r = nc.dram_tensor("w2r", [2, P, 66], dt.bfloat16,
                             kind="ExternalInput")
        identT = nc.dram_tensor("identT", [P, P], dt.bfloat16,
                                kind="ExternalInput")
        outT2 = nc.dram_tensor("outT2", [SHARD, 68], dt.bfloat16,
                               kind="ExternalOutput")
    else:
        outT = nc.dram_tensor("outT", [SHARD, COUT], dt.float32,
                              kind="ExternalOutput")
    idxT = nc.dram_tensor("idxT", [P, Stot], dt.int16, kind="ExternalInput")
    dstlT = nc.dram_tensor("dstlT", [P, Ttot], dt.bfloat16, kind="ExternalInput")
    dstlR = nc.dram_tensor("dstlR", [NB, Tmax * P], dt.bfloat16,
                           kind="ExternalInput")
    adstT = nc.dram_tensor("adstT", [P, NB * H], dt.bfloat16,
                           kind="ExternalInput")
    brow = nc.dram_tensor("brow", [1, COUT], dt.float32, kind="ExternalInput")
    iot_r = nc.dram_tensor("iot_r", [1, P], dt.bfloat16, kind="ExternalInput")
    iot_c = nc.dram_tensor("iot_c", [P, 1], dt.float32, kind="ExternalInput")
    htab = nc.dram_tensor("htab", [NPAD, TABC], dt.bfloat16,
                          kind="ExternalInput")

    SP = bool(int(os.environ.get("GAT_SP", "0")))
    PBB = int(os.environ.get("GAT_PBB", "4"))
    PPK = int(os.environ.get("GAT_PPK", "3"))
    PBM = int(os.environ.get("GAT_PB_MOD%d" % layer,
                             "0" if layer == 1 else "2"))
    GLB = int(os.environ.get("GAT_GLB", "2"))      # gather-B lookahead
    ALATE = bool(int(os.environ.get("GAT_ALATE", "0")))
    HPS = int(os.environ.get("GAT_HPS", "2"))      # hp split count

    with tile.TileContext(nc) as tc:
        with (
            tc.tile_pool(name="const", bufs=1) as cp,
            tc.tile_pool(name="keep", bufs=1) as kp,
            tc.tile_pool(name="pp", bufs=3) as ppool,
            tc.tile_pool(name="pb", bufs=PBB) as pb,
            tc.tile_pool(name="sm", bufs=3) as sm,
            tc.tile_pool(name="psA", bufs=1, space="PSUM") as psA,
            tc.tile_pool(name="psB", bufs=2, space="PSUM") as psB,
            tc.tile_pool(name="psD", bufs=PPK + 1, space="PSUM") as psD,
        ):
            # ---- resident constants ----
            b_sb = cp.tile([P, COUT], dt.float32)
            nc.sync.dma_start(b_sb[:], brow[:].broadcast_to([P, COUT]))
            ior_sb = cp.tile([P, P], dt.bfloat16)
            nc.sync.dma_start(ior_sb[:], iot_r[:].broadcast_to([P, P]))
            ioc_sb = cp.tile([P, 1], dt.float32)
            nc.sync.dma_start(ioc_sb[:], iot_c[:])
            if layer == 1:
                w2_sb = cp.tile([P, 2, 66], dt.bfloat16)
                nc.sync.dma_start(w2_sb[:], w2r[:].rearrange("k p c -> p k c"))
                id_sb = cp.tile([P, P], dt.bfloat16)
                nc.sync.dma_start(id_sb[:], identT[:])
            idx_sb = kp.tile([P, Stot], dt.int16)
            nc.sync.dma_start(idx_sb[:], idxT[:])
            dstl_sb = kp.tile([P, Ttot], dt.bfloat16)
            nc.sync.dma_start(dstl_sb[:], dstlT[:])
            adst_sh = kp.tile([P, NB * H], dt.bfloat16)
            nc.sync.dma_start(adst_sh[:], adstT[:])

            # ---- pre-pass: expand adst to per-edge values (PSUM-resident),
            # staged so the dlr broadcast has a full iteration to land ----
            adst_ps = [None] * NB
            dlr_sb = [None] * NB

            def dlr_issue(b):
                T = Tm[b]
                dlr = ppool.tile([P, T * P], dt.bfloat16, tag="dlr")
                if PBM and b % PBM:
                    dlrow = ppool.tile([1, T * P], dt.bfloat16, tag="dlrow")
                    nc.sync.dma_start(dlrow[:], dstlR[b:b + 1, 0:T * P])
                    nc.gpsimd.partition_broadcast(dlr[:], dlrow[:])
                else:
                    nc.sync.dma_start(
                        dlr[:],
                        dstlR[b:b + 1, 0:T * P].broadcast_to([P, T * P]))
                dlr_sb[b] = dlr

            def prepass_block(b):
                T = Tm[b]
                mT = ppool.tile([P, T, P], dt.bfloat16, tag="mT")
                nc.vector.tensor_scalar(
                    mT[:].rearrange("p t e -> p (t e)"), dlr_sb[b][:],
                    ioc_sb[:], None, mybir.AluOpType.is_equal)
                dlr_sb[b] = None
                ap_ps = psD.tile([P, T * H], dt.float32, tag="adps")
                for t in range(T):
                    nc.tensor.matmul(ap_ps[:, t * H:(t + 1) * H],
                                     lhsT=mT[:, t, :],
                                     rhs=adst_sh[:, b * H:(b + 1) * H],
                                     start=True, stop=True)
                adst_ps[b] = ap_ps

            # ---- gathers: B first (covers tail incl. boundary dummies),
            # then A overwrites its region ----
            htabA = htab[0:SPLIT, :]
            htabB = htab[SPLIT:NPAD, :]

            g_sb = [None] * NB
            mk_sb = [None] * NB

            def issue_gatherB(b):
                T = Tm[b]
                g = pb.tile([P, T, TABC], dt.bfloat16, tag="gath")
                sB = scol[b] + niA16[b] // 16
                if niB[b] > 0:
                    nc.gpsimd.dma_gather(
                        g[:, fAm[b]:T, :], htabB,
                        idx_sb[:, sB:sB + niB[b] // 16],
                        niB[b], niB[b], TABC, single_packet=SP)
                g_sb[b] = g

            def issue_gatherA(b):
                T = Tm[b]
                g = g_sb[b]
                sA = scol[b]
                if niA16[b] > 0:
                    a_tiles = -(-niA16[b] // P)
                    nc.gpsimd.dma_gather(
                        g[:, 0:a_tiles, :], htabA,
                        idx_sb[:, sA:sA + niA16[b] // 16],
                        niA16[b], niA16[b], TABC, single_packet=SP)

            def build_masks(b):
                T = Tm[b]
                # dst one-hot masks (pair-packed for DVE 2x)
                dl2 = sm.tile([P, T, 2], dt.bfloat16, tag="dl2")
                nc.vector.tensor_copy(
                    dl2[:],
                    dstl_sb[:, toff[b]:toff[b] + T].rearrange(
                        "p (t o) -> p t o", o=1).broadcast_to([P, T, 2]))
                mk = sm.tile([P, T, P], dt.bfloat16, tag="mk")   # [e_p,(t,d)]
                nc.vector.tensor_tensor(
                    mk[:].rearrange("p t (d2 pr) -> p t d2 pr", pr=2),
                    ior_sb[:].rearrange("p (t d2 pr) -> p t d2 pr", t=1, pr=2
                                        ).broadcast_to([P, T, P // 2, 2]),
                    dl2[:].rearrange("p t (d2 pr) -> p t d2 pr", d2=1
                                     ).broadcast_to([P, T, P // 2, 2]),
                    mybir.AluOpType.is_equal)
                mk_sb[b] = mk

            def epilogue(b, agg):
                # out = num/(den+eps) (+bias) (+ELU and fused proj, layer 1)
                dn = sm.tile([P, H], dt.float32, tag="dn")
                nc.vector.tensor_scalar_add(dn[:], agg[:, COUT:AGC], 1e-16)
                rc = sm.tile([P, H], dt.float32, tag="rc")
                nc.vector.reciprocal(rc[:], dn[:])
                if layer == 1:
                    ob = sm.tile([P, COUT], dt.bfloat16, tag="ob")
                    for h in range(H):
                        nc.scalar.activation(ob[:, h * F:(h + 1) * F],
                                             agg[:, h * F:(h + 1) * F],
                                             mybir.ActivationFunctionType.Copy,
                                             scale=rc[:, h:h + 1])
                    if not zero_bias:
                        nc.vector.tensor_add(
                            ob[:], ob[:],
                            b_sb[:].bitcast(dt.bfloat16)[:, 1::2])
                    # elu(y) = relu(y) + exp(min(y,0)) - 1
                    r1 = sm.tile([P, COUT], dt.bfloat16, tag="r1")
                    nc.scalar.activation(r1[:], ob[:],
                                         mybir.ActivationFunctionType.Relu,
                                         scale=-1.0)
                    r2 = sm.tile([P, COUT], dt.bfloat16, tag="r2")
                    nc.scalar.activation(r2[:], r1[:],
                                         mybir.ActivationFunctionType.Exp,
                                         scale=-1.0)
                    nc.scalar.activation(ob[:], ob[:],
                                         mybir.ActivationFunctionType.Relu)
                    nc.vector.scalar_tensor_tensor(
                        ob[:], r2[:], -1.0, ob[:],
                        mybir.AluOpType.add, mybir.AluOpType.add)
                    # fused layer-2 row production:
                    # [elu(out1) @ [W2|wasrc2|wadst2]] -> [h2|asrc2|adst2]
                    ps_t = psA.tile([P, 2, P], dt.bfloat16, tag="pst")
                    for c in range(2):
                        nc.tensor.transpose(ps_t[:, c, :],
                                            ob[:, c * P:(c + 1) * P],
                                            id_sb[:])
                    x2T = sm.tile([P, 2, P], dt.bfloat16, tag="x2T")
                    nc.scalar.activation(x2T[:], ps_t[:],
                                         mybir.ActivationFunctionType.Copy)
                    ps2 = psA.tile([P, 66], dt.float32, tag="ps2")
                    for c in range(2):
                        nc.tensor.matmul(ps2[:], lhsT=x2T[:, c, :],
                                         rhs=w2_sb[:, c, :],
                                         start=(c == 0), stop=(c == 1))
                    hst2 = sm.tile([P, 68], dt.bfloat16, tag="hst2")
                    nc.scalar.activation(hst2[:, 0:64], ps2[:, 0:64],
                                         mybir.ActivationFunctionType.Copy)
                    nc.vector.tensor_copy(
                        hst2[:, 64:68].bitcast(dt.float32), ps2[:, 64:66])
                    nc.sync.dma_start(outT2[b * P:(b + 1) * P, :], hst2[:])
                else:
                    ob = sm.tile([P, COUT], dt.float32, tag="ob")
                    nc.scalar.activation(ob[:], agg[:, 0:COUT],
                                         mybir.ActivationFunctionType.Copy,
                                         scale=rc[:, 0:1])
                    if not zero_bias:
                        nc.vector.tensor_add(ob[:], ob[:], b_sb[:])
                    nc.sync.dma_start(outT[b * P:(b + 1) * P, :], ob[:])

            # ---- phase B: software-pipelined per-block message passing.
            # Emission order is tuned for the in-order engine queues: the
            # et->prelu->exp->hp critical chain leads, lookahead issues fill
            # the ACT round-trip, the lagged epilogue never blocks it. ----
            for q in range(min(PPK, NB)):
                dlr_issue(q)
            for q in range(min(PPK - 1, NB)):
                prepass_block(q)
            for q in range(min(GLB, NB)):
                issue_gatherB(q)
            issue_gatherA(0)
            build_masks(0)
            pend = None                     # (block, agg) awaiting epilogue
            for b in range(NB):
                T = Tm[b]
                g = g_sb[b]
                mk = mk_sb[b]

                # w2 = exp(prelu(asrc + adst)) pair-broadcast, on ACT
                et = sm.tile([P, T * H], dt.float32, tag="et")
                nc.vector.tensor_tensor(
                    et[:].rearrange("p (t h) -> p t h", h=H),
                    g[:, :, COUT:COUT + 2 * H].bitcast(dt.float32),
                    adst_ps[b][:].rearrange("p (t h) -> p t h", h=H),
                    mybir.AluOpType.add)
                adst_ps[b] = None
                lr = sm.tile([P, T * H], dt.float32, tag="lr")
                nc.scalar.activation(lr[:], et[:],
                                     mybir.ActivationFunctionType.Prelu,
                                     alpha=NEG)
                wt2 = sm.tile([P, T, H, 2], dt.bfloat16, tag="wt2")
                nc.scalar.activation(
                    wt2[:],
                    lr[:].rearrange("p (t h o) -> p t h o", h=H, o=1
                                    ).broadcast_to([P, T, H, 2]),
                    mybir.ActivationFunctionType.Exp)

                # lookahead issues (fill the ACT round-trip gap on DVE/Pool)
                if not ALATE and b + 1 < NB:
                    issue_gatherA(b + 1)
                if b + PPK < NB:
                    dlr_issue(b + PPK)
                if b + PPK - 1 < NB:
                    prepass_block(b + PPK - 1)
                if b + GLB < NB:
                    issue_gatherB(b + GLB)
                if pend is not None:
                    epilogue(*pend)
                    pend = None
                if b + 1 < NB:
                    if ALATE:
                        issue_gatherA(b + 1)
                    build_masks(b + 1)

                # hp = [w * h | w]  (pair-packed 2x multiply, split in halves
                # so aggregation can start on the first half early)
                hp = sm.tile([P, T, AGC], dt.bfloat16, tag="hp")
                agg = psB.tile([P, AGC], dt.float32, tag="agg")
                nhs = min(HPS, T)
                bnds = [t * T // nhs for t in range(nhs)] + [T]
                for t0, t1 in zip(bnds[:-1], bnds[1:]):
                    if t0 >= t1:
                        continue
                    ts = t1 - t0
                    nc.vector.tensor_tensor(
                        hp[:, t0:t1, 0:COUT].rearrange(
                            "p t (h f2 pr) -> p t h f2 pr", h=H, pr=2),
                        g[:, t0:t1, 0:COUT].rearrange(
                            "p t (h f2 pr) -> p t h f2 pr", h=H, pr=2),
                        wt2[:, t0:t1].rearrange(
                            "p t (h1 h) pr -> p t h h1 pr", h1=1
                        ).broadcast_to([P, ts, H, F // 2, 2]),
                        mybir.AluOpType.mult)
                    nc.vector.tensor_copy(
                        hp[:, t0:t1, COUT:AGC],
                        wt2[:, t0:t1, :, 0])
                    for t in range(t0, t1):
                        nc.tensor.matmul(agg[:], lhsT=mk[:, t, :],
                                         rhs=hp[:, t, :],
                                         start=(t == 0), stop=(t == T - 1))
                g_sb[b] = None
                mk_sb[b] = None
                pend = (b, agg)
            epilogue(*pend)

    nc.compile()
    return nc


# ----------------------------------------------------------------------------
# host-side weight packing
# ----------------------------------------------------------------------------

def _expand_att(att, H, F):
    out = np.zeros((H * F, H), np.float32)
    for h in range(H):
        out[h * F:(h + 1) * F, h] = att[h]
    return out


def _inputs_layer(meta, idx_all, dstl_pt, dstl_row, b, layer):
    H = H1 if layer == 1 else 1
    COUT = H * F1
    b_np = np.asarray(b, np.float32).reshape(1, COUT)
    ior = np.arange(P, dtype=np.float32).reshape(1, P).astype(BF16)
    ioc = np.arange(P, dtype=np.float32).reshape(P, 1)
    in_maps = []
    for m in range(NCORES):
        in_maps.append({
            "idxT": idx_all[m],
            "dstlT": dstl_pt[m].astype(BF16),
            "dstlR": dstl_row[m],
            "brow": b_np, "iot_r": ior, "iot_c": ioc,
        })
    return in_maps


# ----------------------------------------------------------------------------
# entry point
# ----------------------------------------------------------------------------

def kernel(x, edge_index, W1, att_src1, att_dst1, b1, W2, att_src2, att_dst2,
           b2):
    x = np.asarray(x, np.float32)
    edge_index = np.asarray(edge_index)

    meta, idx_all, dstl_pt, dstl_row = _prep_edges(edge_index)
    asg = np.asarray(meta["asg"])                     # [NB, NCORES]

    # ---- launch 0: per-node projection [h | asrc | adst] ----
    key0 = (0,)
    if key0 not in _prog_cache:
        _prog_cache[key0] = _build_proj()
    nc0 = _prog_cache[key0]

    W1f = np.asarray(W1, np.float32)
    wasrc1 = W1f @ _expand_att(np.asarray(att_src1, np.float32), H1, F1)
    wadst1 = W1f @ _expand_att(np.asarray(att_dst1, np.float32), H1, F1)
    wr_np = np.concatenate([W1f, wasrc1, wadst1], axis=1)
    wr_np = np.ascontiguousarray(wr_np.reshape(1, P, 256 + 2 * H1)).astype(BF16)

    xpad = np.zeros((NCORES * SHARD, IN), np.float32)
    xpad[:N] = x
    in_maps0 = []
    for m in range(NCORES):
        shard = xpad[m * SHARD:(m + 1) * SHARD]
        xs_np = np.ascontiguousarray(
            shard.reshape(NB, P, 1, P).transpose(2, 3, 0, 1)).astype(BF16)
        in_maps0.append({"xs": xs_np, "wr": wr_np})
    res0 = run_bass_kernel_spmd(nc0, in_maps0, list(range(NCORES))).results

    COUT1 = H1 * F1
    htab1 = np.zeros((NPAD, 384), BF16)
    adst1 = np.zeros((N + P, H1), np.float32)         # per-node adst (layer 1)
    for m in range(NCORES):
        lo = m * SHARD
        hi = min(NPAD, (m + 1) * SHARD)
        hts = res0[m]["hts"][:hi - lo]
        htab1[lo:hi, 0:COUT1 + 2 * H1] = hts[:, 0:COUT1 + 2 * H1]
        adst1[lo:hi] = hts[:, COUT1 + 2 * H1:].copy().view(np.float32)

    # per-core adst in assigned-block order: [P, NB*H] (partition = dst local)
    def adst_input(adst_n, H):
        out = []
        for m in range(NCORES):
            a = np.zeros((NB, P, H), np.float32)
            for i in range(NB):
                g = asg[i, m]
                rows = adst_n[g * P:(g + 1) * P]
                a[i, :len(rows)] = rows
            out.append(np.ascontiguousarray(
                a.transpose(1, 0, 2).reshape(P, NB * H)).astype(BF16))
        return out

    # ---- layer 1 ----
    tkey = tuple(meta["Tm"])
    zb1 = bool(np.all(np.asarray(b1) == 0))
    key1 = (1, tkey, zb1)
    if key1 not in _prog_cache:
        _prog_cache[key1] = _build_layer(meta, 1, zb1)
    ncA = _prog_cache[key1]
    in_maps = _inputs_layer(meta, idx_all, dstl_pt, dstl_row, b1, 1)
    W2f = np.asarray(W2, np.float32)
    wasrc2 = W2f @ np.asarray(att_src2, np.float32).reshape(F1, 1)
    wadst2 = W2f @ np.asarray(att_dst2, np.float32).reshape(F1, 1)
    w2r_np = np.ascontiguousarray(
        np.concatenate([W2f, wasrc2, wadst2], axis=1).reshape(2, P, 66)
    ).astype(BF16)
    ident = np.eye(P, dtype=np.float32).astype(BF16)
    a1in = adst_input(adst1, H1)
    for m, mmap in enumerate(in_maps):
        mmap["w2r"] = w2r_np
        mmap["identT"] = ident
        mmap["htab"] = htab1
        mmap["adstT"] = a1in[m]
    resA = run_bass_kernel_spmd(ncA, in_maps, list(range(NCORES))).results

    # reassemble layer-2 table + adst2 from assigned-block outputs
    htab2 = np.zeros((NPAD, 128), BF16)
    adst2 = np.zeros((N + P, 1), np.float32)
    for m in range(NCORES):
        o2 = resA[m]["outT2"]
        for i in range(NB):
            g = asg[i, m]
            if g * P >= NPAD:
                continue
            hi = min(NPAD, (g + 1) * P) - g * P
            htab2[g * P:g * P + hi, 0:66] = o2[i * P:i * P + hi, 0:66]
            adst2[g * P:g * P + hi, 0] = (
                o2[i * P:i * P + hi, 66:68].copy().view(np.float32)[:, 0])

    # ---- layer 2 ----
    zb2 = bool(np.all(np.asarray(b2) == 0))
    key2 = (2, tkey, zb2)
    if key2 not in _prog_cache:
        _prog_cache[key2] = _build_layer(meta, 2, zb2)
    ncB = _prog_cache[key2]
    in_maps2 = _inputs_layer(meta, idx_all, dstl_pt, dstl_row, b2, 2)
    a2in = adst_input(adst2, 1)
    for m, mmap in enumerate(in_maps2):
        mmap["htab"] = htab2
        mmap["adstT"] = a2in[m]
    resB = run_bass_kernel_spmd(ncB, in_maps2, list(range(NCORES))).results

    out = np.zeros((N, F1), np.float32)
    for m in range(NCORES):
        o = resB[m]["outT"]
        for i in range(NB):
            g = asg[i, m]
            lo = g * P
            if lo >= N:
                continue
            hi = min(N, lo + P)
            out[lo:hi] = o[i * P:i * P + (hi - lo)].astype(np.float32)
    return out


# revision 17
# speedup vs baseline: 1.1554x; 1.0679x over previous
"""2-layer GAT on 8 Trainium2 NeuronCores (Bass/Tile).

Sharding: the 391 dst 128-node blocks are sorted by half-A edge count and
dealt in groups of 8 to the cores (one block per core per iteration), so the
per-iteration cross-core tile maximum stays near the mean.  Edges are routed
to the core owning their dst block and laid out in shared tiles: half-A rows
(table rows < SPLIT, int16-indexable) first, padded to the iteration max,
then half-B rows in the same tile array (gather B runs first with leading
dummy indices, gather A then overwrites its region).

Per-layer device program (phase B only; projections are fused elsewhere):
  per 128-dst block: dma_gather B + A from the HBM row table
  [h | asrc f32-bits], one-hot dst masks on DVE (pair-packed 2x),
  per-edge adst via maskT matmuls (PSUM-resident), w = exp(prelu(asrc+adst))
  on ACT, weighted rows on DVE, aggregation + softmax denominator via
  PSUM-accumulated matmuls, epilogue scaling on ACT.

Launch 0 projects [h | asrc | adst] per node; layer 1's epilogue fuses the
layer-2 projection [h2 | asrc2 | adst2], so neither layer loads x at all.
Shards are exchanged through the host between launches.
"""

import os
import numpy as np
import ml_dtypes

import concourse.bass as bass
import concourse.bacc as bacc
import concourse.tile as tile
from concourse import mybir
from concourse.bass_utils import run_bass_kernel_spmd

BF16 = ml_dtypes.bfloat16

N = 50000
E = 800000
IN = 128
H1 = 4
F1 = 64
NEG = 0.2
P = 128
NCORES = 8
NB = 49                 # block iterations per core
SHARD = NB * P          # 6272 rows per core in the table
NPAD = 391 * P          # 50048 padded node count
NGB = NCORES * NB       # 392 block slots (391 real + 1 dummy)
SPLIT = 196 * P         # 25088: gather-table half boundary (int16 idx limit)
GRP = 16                # proj-launch load group

_prog_cache = {}


# ----------------------------------------------------------------------------
# host-side edge preprocessing (shared by both layers)
# ----------------------------------------------------------------------------

def _prep_edges(edge_index):
    src = np.concatenate([edge_index[0].astype(np.int64), np.arange(N, dtype=np.int64)])
    dst = np.concatenate([edge_index[1].astype(np.int64), np.arange(N, dtype=np.int64)])
    order = np.argsort(dst, kind="stable")
    s = src[order]
    d = dst[order]

    gb = d >> 7                                   # global 128-block of dst
    cnt = np.bincount(gb, minlength=NGB)
    starts = np.concatenate([[0], np.cumsum(cnt)])
    isB = s >= SPLIT
    cntA = np.zeros(NGB, np.int64)
    for g in range(NGB):
        cntA[g] = np.count_nonzero(~isB[starts[g]:starts[g + 1]])

    # deal blocks sorted by half-A count: iteration i gets ranks [8i, 8i+8)
    blk_order = np.argsort(-cntA, kind="stable")
    asg = blk_order.reshape(NB, NCORES)           # [iter, core] -> global block
    nA = cntA[asg]                                # [NB, NCORES]
    nBc = (cnt - cntA)[asg]
    nAmax = nA.max(1)                             # [NB]
    nBmax = nBc.max(1)
    Tm = np.maximum(-(-(nAmax + nBmax) // P), 1).astype(np.int64)
    niA16 = (-(-nAmax // 16) * 16).astype(np.int64)   # static gather-A num_idxs
    fA = nAmax // P                               # full A tiles
    rA = nAmax % P                                # B's leading dummy count
    niB = (Tm - fA) * P                           # static gather-B num_idxs

    toff = np.zeros(NB + 1, np.int64)
    np.cumsum(Tm, out=toff[1:])
    Ttot = int(toff[NB])
    TMX = int(Tm.max())

    scol = np.zeros(NB + 1, np.int64)             # idx column offsets (per 16)
    np.cumsum(niA16 // 16 + niB // 16, out=scol[1:])
    Stot = int(scol[NB])

    idx_all = np.zeros((NCORES, P, Stot), np.int16)
    dstl = np.full((NCORES, Ttot, P), -1.0, np.float32)   # [t, p] layout

    for i in range(NB):
        sA = int(scol[i])
        sB = sA + int(niA16[i] // 16)
        for m in range(NCORES):
            g = asg[i, m]
            e0, e1 = starts[g], starts[g + 1]
            sb = s[e0:e1]
            mB = isB[e0:e1]
            shA = sb[~mB]
            shB = sb[mB] - SPLIT
            dlA = (d[e0:e1][~mB] - (g << 7)).astype(np.float32)
            dlB = (d[e0:e1][mB] - (g << 7)).astype(np.float32)
            na, nb_ = len(shA), len(shB)
            # gather-A idx: real | dummy-0 to nAmax | -1 tail to niA16
            ia = np.zeros(int(niA16[i]), np.int16)
            ia[:na] = shA
            ia[int(nAmax[i]):] = -1
            # gather-B idx: rA dummy-0 | real | dummy-0 tail
            ib = np.zeros(int(niB[i]), np.int16)
            ib[int(rA[i]):int(rA[i]) + nb_] = shB
            for seg, off in ((ia, sA), (ib, sB)):
                w = seg.reshape(-1, 16).T          # [16, S]
                idx_all[m][:, off:off + w.shape[1]] = np.tile(w, (8, 1))
            dl = np.full(int(Tm[i]) * P, -1.0, np.float32)
            dl[:na] = dlA
            dl[int(nAmax[i]):int(nAmax[i]) + nb_] = dlB
            dstl[m][toff[i]:toff[i] + Tm[i], :] = dl.reshape(int(Tm[i]), P)

    dstl_pt = np.ascontiguousarray(dstl.transpose(0, 2, 1))   # [m, P, Ttot]
    dstl_row = np.full((NCORES, NB, TMX * P), -1.0, np.float32)
    for m in range(NCORES):
        for i in range(NB):
            T = int(Tm[i])
            dstl_row[m, i, :T * P] = dstl[m, toff[i]:toff[i] + T].reshape(-1)
    dstl_row = dstl_row.astype(BF16)

    meta = dict(Tm=Tm.tolist(), toff=toff.tolist(), fA=fA.tolist(),
                rA=rA.tolist(), niA16=niA16.tolist(), niB=niB.tolist(),
                scol=scol.tolist(), Ttot=Ttot, Stot=Stot, Tmax=TMX,
                asg=asg.tolist())
    return meta, idx_all, dstl_pt, dstl_row


# ----------------------------------------------------------------------------
# launch 0: project own shard -> [h | asrc | adst] table slice
# ----------------------------------------------------------------------------

def _build_proj():
    dt = mybir.dt
    KCH, H = 1, H1
    COUT = H1 * F1
    RC = COUT + 2 * H                   # [h | asrc | adst]
    OCOL = COUT + 4 * H                 # bf16 slots: h | asrc bits | adst bits
    nc = bacc.Bacc("TRN2", target_bir_lowering=False, debug=False,
                   num_devices=NCORES)
    xs = nc.dram_tensor("xs", [KCH, P, NB, P], dt.bfloat16,
                        kind="ExternalInput")
    wr = nc.dram_tensor("wr", [KCH, P, RC], dt.bfloat16,
                        kind="ExternalInput")
    hts = nc.dram_tensor("hts", [SHARD, OCOL], dt.bfloat16,
                         kind="ExternalOutput")
    with tile.TileContext(nc) as tc:
        with (
            tc.tile_pool(name="const", bufs=1) as cp,
            tc.tile_pool(name="pa", bufs=3) as pa,
            tc.tile_pool(name="psA", bufs=3, space="PSUM") as psA,
        ):
            wr_sb = cp.tile([P, KCH, RC], dt.bfloat16)
            nc.sync.dma_start(wr_sb[:], wr[:].rearrange("k p c -> p k c"))
            for g0 in range(0, NB, GRP):
                gn = min(GRP, NB - g0)
                xa = pa.tile([P, KCH, gn, P], dt.bfloat16, tag="xa")
                nc.sync.dma_start(
                    xa[:], xs[:, :, g0:g0 + gn, :].rearrange(
                        "k f t n -> f k t n"))
                hst = pa.tile([P, gn, OCOL], dt.bfloat16, tag="hst")
                for t0 in range(0, gn, 2):
                    pn = min(2, gn - t0)
                    ps = psA.tile([P, 2, 512], dt.float32, tag="psa")
                    for t2 in range(pn):
                        for k in range(KCH):
                            nc.tensor.matmul(ps[:, t2, 0:RC],
                                             lhsT=xa[:, k, t0 + t2, :],
                                             rhs=wr_sb[:, k, 0:RC],
                                             start=(k == 0),
                                             stop=(k == KCH - 1))
                    nc.scalar.activation(
                        hst[:, t0:t0 + pn, 0:COUT], ps[:, 0:pn, 0:COUT],
                        mybir.ActivationFunctionType.Copy)
                    nc.vector.tensor_copy(
                        hst[:, t0:t0 + pn, COUT:OCOL].bitcast(dt.float32),
                        ps[:, 0:pn, COUT:COUT + 2 * H])
                nc.sync.dma_start(
                    hts[g0 * P:(g0 + gn) * P, :].rearrange(
                        "(t n) c -> n t c", t=gn),
                    hst[:])
    nc.compile()
    return nc


# ----------------------------------------------------------------------------
# per-layer message-passing program (phase B)
# ----------------------------------------------------------------------------

def _build_layer(meta, layer, zero_bias):
    """layer 1: heads 4, F 64, fused layer-2 row production, no dense out.
    layer 2: heads 1, F 64, out f32 [SHARD, 64]."""
    dt = mybir.dt
    Tm, toff, fAm = meta["Tm"], meta["toff"], meta["fA"]
    niA16, niB, scol = meta["niA16"], meta["niB"], meta["scol"]
    Ttot, Stot, Tmax = meta["Ttot"], meta["Stot"], meta["Tmax"]

    if layer == 1:
        H, F = H1, F1
    else:
        H, F = 1, F1
    COUT = H * F
    AGC = COUT + H                    # aggregation psum cols: [num | den]
    TABC = 384 if layer == 1 else 128  # table row slots (256B granules)

    nc = bacc.Bacc("TRN2", target_bir_lowering=False, debug=False,
                   num_devices=NCORES)

    if layer == 1:
        w2r = nc.dram_tensor("w2r", [2, P, 66], dt.bfloat16,
                             kind="ExternalInput")
        identT = nc.dram_tensor("identT", [P, P], dt.bfloat16,
                                kind="ExternalInput")
        outT2 = nc.dram_tensor("outT2", [SHARD, 68], dt.bfloat16,
                               kind="ExternalOutput")
    else:
        outT = nc.dram_tensor("outT", [SHARD, COUT], dt.float32,
                              kind="ExternalOutput")
    idxT = nc.dram_tensor("idxT", [P, Stot], dt.int16, kind="ExternalInput")
    dstlT = nc.dram_tensor("dstlT", [P, Ttot], dt.bfloat16, kind="ExternalInput")
    dstlR = nc.dram_tensor("dstlR", [NB, Tmax * P], dt.bfloat16,
                           kind="ExternalInput")
    adstT = nc.dram_tensor("adstT", [P, NB * H], dt.bfloat16,
                           kind="ExternalInput")
    brow = nc.dram_tensor("brow", [1, COUT], dt.float32, kind="ExternalInput")
    iot_r = nc.dram_tensor("iot_r", [1, P], dt.bfloat16, kind="ExternalInput")
    iot_c = nc.dram_tensor("iot_c", [P, 1], dt.float32, kind="ExternalInput")
    htab = nc.dram_tensor("htab", [NPAD, TABC], dt.bfloat16,
                          kind="ExternalInput")

    SP = bool(int(os.environ.get("GAT_SP", "0")))
    PBB = int(os.environ.get("GAT_PBB", "4"))
    PPK = int(os.environ.get("GAT_PPK", "3"))
    PBM = int(os.environ.get("GAT_PB_MOD%d" % layer,
                             "0" if layer == 1 else "2"))
    GLB = int(os.environ.get("GAT_GLB", "2"))      # gather-B lookahead
    ALATE = bool(int(os.environ.get("GAT_ALATE", "1")))
    HPS = int(os.environ.get("GAT_HPS", "2"))      # hp split count

    with tile.TileContext(nc) as tc:
        with (
            tc.tile_pool(name="const", bufs=1) as cp,
            tc.tile_pool(name="keep", bufs=1) as kp,
            tc.tile_pool(name="pp", bufs=3) as ppool,
            tc.tile_pool(name="pb", bufs=PBB) as pb,
            tc.tile_pool(name="sm", bufs=3) as sm,
            tc.tile_pool(name="psA", bufs=1, space="PSUM") as psA,
            tc.tile_pool(name="psB", bufs=2, space="PSUM") as psB,
            tc.tile_pool(name="psD", bufs=PPK + 1, space="PSUM") as psD,
        ):
            # ---- resident constants ----
            b_sb = cp.tile([P, COUT], dt.float32)
            nc.sync.dma_start(b_sb[:], brow[:].broadcast_to([P, COUT]))
            ior_sb = cp.tile([P, P], dt.bfloat16)
            nc.sync.dma_start(ior_sb[:], iot_r[:].broadcast_to([P, P]))
            ioc_sb = cp.tile([P, 1], dt.float32)
            nc.sync.dma_start(ioc_sb[:], iot_c[:])
            if layer == 1:
                w2_sb = cp.tile([P, 2, 66], dt.bfloat16)
                nc.sync.dma_start(w2_sb[:], w2r[:].rearrange("k p c -> p k c"))
                id_sb = cp.tile([P, P], dt.bfloat16)
                nc.sync.dma_start(id_sb[:], identT[:])
            idx_sb = kp.tile([P, Stot], dt.int16)
            nc.sync.dma_start(idx_sb[:], idxT[:])
            dstl_sb = kp.tile([P, Ttot], dt.bfloat16)
            nc.sync.dma_start(dstl_sb[:], dstlT[:])
            adst_sh = kp.tile([P, NB * H], dt.bfloat16)
            nc.sync.dma_start(adst_sh[:], adstT[:])

            # ---- pre-pass: expand adst to per-edge values (PSUM-resident),
            # staged so the dlr broadcast has a full iteration to land ----
            adst_ps = [None] * NB
            dlr_sb = [None] * NB

            def dlr_issue(b):
                T = Tm[b]
                dlr = ppool.tile([P, T * P], dt.bfloat16, tag="dlr")
                if PBM and b % PBM:
                    dlrow = ppool.tile([1, T * P], dt.bfloat16, tag="dlrow")
                    nc.sync.dma_start(dlrow[:], dstlR[b:b + 1, 0:T * P])
                    nc.gpsimd.partition_broadcast(dlr[:], dlrow[:])
                else:
                    nc.sync.dma_start(
                        dlr[:],
                        dstlR[b:b + 1, 0:T * P].broadcast_to([P, T * P]))
                dlr_sb[b] = dlr

            def prepass_block(b):
                T = Tm[b]
                mT = ppool.tile([P, T, P], dt.bfloat16, tag="mT")
                nc.vector.tensor_scalar(
                    mT[:].rearrange("p t e -> p (t e)"), dlr_sb[b][:],
                    ioc_sb[:], None, mybir.AluOpType.is_equal)
                dlr_sb[b] = None
                ap_ps = psD.tile([P, T * H], dt.float32, tag="adps")
                for t in range(T):
                    nc.tensor.matmul(ap_ps[:, t * H:(t + 1) * H],
                                     lhsT=mT[:, t, :],
                                     rhs=adst_sh[:, b * H:(b + 1) * H],
                                     start=True, stop=True)
                adst_ps[b] = ap_ps

            # ---- gathers: B first (covers tail incl. boundary dummies),
            # then A overwrites its region ----
            htabA = htab[0:SPLIT, :]
            htabB = htab[SPLIT:NPAD, :]

            g_sb = [None] * NB
            mk_sb = [None] * NB

            def issue_gatherB(b):
                T = Tm[b]
                g = pb.tile([P, T, TABC], dt.bfloat16, tag="gath")
                sB = scol[b] + niA16[b] // 16
                if niB[b] > 0:
                    nc.gpsimd.dma_gather(
                        g[:, fAm[b]:T, :], htabB,
                        idx_sb[:, sB:sB + niB[b] // 16],
                        niB[b], niB[b], TABC, single_packet=SP)
                g_sb[b] = g

            def issue_gatherA(b):
                T = Tm[b]
                g = g_sb[b]
                sA = scol[b]
                if niA16[b] > 0:
                    a_tiles = -(-niA16[b] // P)
                    nc.gpsimd.dma_gather(
                        g[:, 0:a_tiles, :], htabA,
                        idx_sb[:, sA:sA + niA16[b] // 16],
                        niA16[b], niA16[b], TABC, single_packet=SP)

            def build_masks(b):
                T = Tm[b]
                # dst one-hot masks (pair-packed for DVE 2x)
                dl2 = sm.tile([P, T, 2], dt.bfloat16, tag="dl2")
                nc.vector.tensor_copy(
                    dl2[:],
                    dstl_sb[:, toff[b]:toff[b] + T].rearrange(
                        "p (t o) -> p t o", o=1).broadcast_to([P, T, 2]))
                mk = sm.tile([P, T, P], dt.bfloat16, tag="mk")   # [e_p,(t,d)]
                nc.vector.tensor_tensor(
                    mk[:].rearrange("p t (d2 pr) -> p t d2 pr", pr=2),
                    ior_sb[:].rearrange("p (t d2 pr) -> p t d2 pr", t=1, pr=2
                                        ).broadcast_to([P, T, P // 2, 2]),
                    dl2[:].rearrange("p t (d2 pr) -> p t d2 pr", d2=1
                                     ).broadcast_to([P, T, P // 2, 2]),
                    mybir.AluOpType.is_equal)
                mk_sb[b] = mk

            def epilogue(b, agg):
                # out = num/(den+eps) (+bias) (+ELU and fused proj, layer 1)
                dn = sm.tile([P, H], dt.float32, tag="dn")
                nc.vector.tensor_scalar_add(dn[:], agg[:, COUT:AGC], 1e-16)
                rc = sm.tile([P, H], dt.float32, tag="rc")
                nc.vector.reciprocal(rc[:], dn[:])
                if layer == 1:
                    ob = sm.tile([P, COUT], dt.bfloat16, tag="ob")
                    for h in range(H):
                        nc.scalar.activation(ob[:, h * F:(h + 1) * F],
                                             agg[:, h * F:(h + 1) * F],
                                             mybir.ActivationFunctionType.Copy,
                                             scale=rc[:, h:h + 1])
                    if not zero_bias:
                        nc.vector.tensor_add(
                            ob[:], ob[:],
                            b_sb[:].bitcast(dt.bfloat16)[:, 1::2])
                    # elu(y) = relu(y) + exp(min(y,0)) - 1
                    r1 = sm.tile([P, COUT], dt.bfloat16, tag="r1")
                    nc.scalar.activation(r1[:], ob[:],
                                         mybir.ActivationFunctionType.Relu,
                                         scale=-1.0)
                    r2 = sm.tile([P, COUT], dt.bfloat16, tag="r2")
                    nc.scalar.activation(r2[:], r1[:],
                                         mybir.ActivationFunctionType.Exp,
                                         scale=-1.0)
                    nc.scalar.activation(ob[:], ob[:],
                                         mybir.ActivationFunctionType.Relu)
                    nc.vector.scalar_tensor_tensor(
                        ob[:], r2[:], -1.0, ob[:],
                        mybir.AluOpType.add, mybir.AluOpType.add)
                    # fused layer-2 row production:
                    # [elu(out1) @ [W2|wasrc2|wadst2]] -> [h2|asrc2|adst2]
                    ps_t = psA.tile([P, 2, P], dt.bfloat16, tag="pst")
                    for c in range(2):
                        nc.tensor.transpose(ps_t[:, c, :],
                                            ob[:, c * P:(c + 1) * P],
                                            id_sb[:])
                    x2T = sm.tile([P, 2, P], dt.bfloat16, tag="x2T")
                    nc.scalar.activation(x2T[:], ps_t[:],
                                         mybir.ActivationFunctionType.Copy)
                    ps2 = psA.tile([P, 66], dt.float32, tag="ps2")
                    for c in range(2):
                        nc.tensor.matmul(ps2[:], lhsT=x2T[:, c, :],
                                         rhs=w2_sb[:, c, :],
                                         start=(c == 0), stop=(c == 1))
                    hst2 = sm.tile([P, 68], dt.bfloat16, tag="hst2")
                    nc.scalar.activation(hst2[:, 0:64], ps2[:, 0:64],
                                         mybir.ActivationFunctionType.Copy)
                    nc.vector.tensor_copy(
                        hst2[:, 64:68].bitcast(dt.float32), ps2[:, 64:66])
                    nc.sync.dma_start(outT2[b * P:(b + 1) * P, :], hst2[:])
                else:
                    ob = sm.tile([P, COUT], dt.float32, tag="ob")
                    nc.scalar.activation(ob[:], agg[:, 0:COUT],
                                         mybir.ActivationFunctionType.Copy,
                                         scale=rc[:, 0:1])
                    if not zero_bias:
                        nc.vector.tensor_add(ob[:], ob[:], b_sb[:])
                    nc.sync.dma_start(outT[b * P:(b + 1) * P, :], ob[:])

            # ---- phase B: software-pipelined per-block message passing.
            # Emission order is tuned for the in-order engine queues: the
            # et->prelu->exp->hp critical chain leads, lookahead issues fill
            # the ACT round-trip, the lagged epilogue never blocks it. ----
            for q in range(min(PPK, NB)):
                dlr_issue(q)
            for q in range(min(PPK - 1, NB)):
                prepass_block(q)
            for q in range(min(GLB, NB)):
                issue_gatherB(q)
            issue_gatherA(0)
            build_masks(0)
            pend = None                     # (block, agg) awaiting epilogue
            for b in range(NB):
                T = Tm[b]
                g = g_sb[b]
                mk = mk_sb[b]

                # w2 = exp(prelu(asrc + adst)) pair-broadcast, on ACT
                et = sm.tile([P, T * H], dt.float32, tag="et")
                nc.vector.tensor_tensor(
                    et[:].rearrange("p (t h) -> p t h", h=H),
                    g[:, :, COUT:COUT + 2 * H].bitcast(dt.float32),
                    adst_ps[b][:].rearrange("p (t h) -> p t h", h=H),
                    mybir.AluOpType.add)
                adst_ps[b] = None
                lr = sm.tile([P, T * H], dt.float32, tag="lr")
                nc.scalar.activation(lr[:], et[:],
                                     mybir.ActivationFunctionType.Prelu,
                                     alpha=NEG)
                wt2 = sm.tile([P, T, H, 2], dt.bfloat16, tag="wt2")
                nc.scalar.activation(
                    wt2[:],
                    lr[:].rearrange("p (t h o) -> p t h o", h=H, o=1
                                    ).broadcast_to([P, T, H, 2]),
                    mybir.ActivationFunctionType.Exp)

                # lookahead issues (fill the ACT round-trip gap on DVE/Pool)
                if not ALATE and b + 1 < NB:
                    issue_gatherA(b + 1)
                if b + PPK < NB:
                    dlr_issue(b + PPK)
                if b + PPK - 1 < NB:
                    prepass_block(b + PPK - 1)
                if b + GLB < NB:
                    issue_gatherB(b + GLB)
                if pend is not None:
                    epilogue(*pend)
                    pend = None
                if b + 1 < NB:
                    if ALATE:
                        issue_gatherA(b + 1)
                    build_masks(b + 1)

                # hp = [w * h | w]  (pair-packed 2x multiply, split in halves
                # so aggregation can start on the first half early)
                hp = sm.tile([P, T, AGC], dt.bfloat16, tag="hp")
                agg = psB.tile([P, AGC], dt.float32, tag="agg")
                nhs = min(HPS, T)
                bnds = [t * T // nhs for t in range(nhs)] + [T]
                for t0, t1 in zip(bnds[:-1], bnds[1:]):
                    if t0 >= t1:
                        continue
                    ts = t1 - t0
                    nc.vector.tensor_tensor(
                        hp[:, t0:t1, 0:COUT].rearrange(
                            "p t (h f2 pr) -> p t h f2 pr", h=H, pr=2),
                        g[:, t0:t1, 0:COUT].rearrange(
                            "p t (h f2 pr) -> p t h f2 pr", h=H, pr=2),
                        wt2[:, t0:t1].rearrange(
                            "p t (h1 h) pr -> p t h h1 pr", h1=1
                        ).broadcast_to([P, ts, H, F // 2, 2]),
                        mybir.AluOpType.mult)
                    nc.vector.tensor_copy(
                        hp[:, t0:t1, COUT:AGC],
                        wt2[:, t0:t1, :, 0])
                    for t in range(t0, t1):
                        nc.tensor.matmul(agg[:], lhsT=mk[:, t, :],
                                         rhs=hp[:, t, :],
                                         start=(t == 0), stop=(t == T - 1))
                g_sb[b] = None
                mk_sb[b] = None
                pend = (b, agg)
            epilogue(*pend)

    nc.compile()
    return nc


# ----------------------------------------------------------------------------
# host-side weight packing
# ----------------------------------------------------------------------------

def _expand_att(att, H, F):
    out = np.zeros((H * F, H), np.float32)
    for h in range(H):
        out[h * F:(h + 1) * F, h] = att[h]
    return out


def _inputs_layer(meta, idx_all, dstl_pt, dstl_row, b, layer):
    H = H1 if layer == 1 else 1
    COUT = H * F1
    b_np = np.asarray(b, np.float32).reshape(1, COUT)
    ior = np.arange(P, dtype=np.float32).reshape(1, P).astype(BF16)
    ioc = np.arange(P, dtype=np.float32).reshape(P, 1)
    in_maps = []
    for m in range(NCORES):
        in_maps.append({
            "idxT": idx_all[m],
            "dstlT": dstl_pt[m].astype(BF16),
            "dstlR": dstl_row[m],
            "brow": b_np, "iot_r": ior, "iot_c": ioc,
        })
    return in_maps


# ----------------------------------------------------------------------------
# entry point
# ----------------------------------------------------------------------------

def kernel(x, edge_index, W1, att_src1, att_dst1, b1, W2, att_src2, att_dst2,
           b2):
    x = np.asarray(x, np.float32)
    edge_index = np.asarray(edge_index)

    meta, idx_all, dstl_pt, dstl_row = _prep_edges(edge_index)
    asg = np.asarray(meta["asg"])                     # [NB, NCORES]

    # ---- launch 0: per-node projection [h | asrc | adst] ----
    key0 = (0,)
    if key0 not in _prog_cache:
        _prog_cache[key0] = _build_proj()
    nc0 = _prog_cache[key0]

    W1f = np.asarray(W1, np.float32)
    wasrc1 = W1f @ _expand_att(np.asarray(att_src1, np.float32), H1, F1)
    wadst1 = W1f @ _expand_att(np.asarray(att_dst1, np.float32), H1, F1)
    wr_np = np.concatenate([W1f, wasrc1, wadst1], axis=1)
    wr_np = np.ascontiguousarray(wr_np.reshape(1, P, 256 + 2 * H1)).astype(BF16)

    xpad = np.zeros((NCORES * SHARD, IN), np.float32)
    xpad[:N] = x
    in_maps0 = []
    for m in range(NCORES):
        shard = xpad[m * SHARD:(m + 1) * SHARD]
        xs_np = np.ascontiguousarray(
            shard.reshape(NB, P, 1, P).transpose(2, 3, 0, 1)).astype(BF16)
        in_maps0.append({"xs": xs_np, "wr": wr_np})
    res0 = run_bass_kernel_spmd(nc0, in_maps0, list(range(NCORES))).results

    COUT1 = H1 * F1
    htab1 = np.zeros((NPAD, 384), BF16)
    adst1 = np.zeros((N + P, H1), np.float32)         # per-node adst (layer 1)
    for m in range(NCORES):
        lo = m * SHARD
        hi = min(NPAD, (m + 1) * SHARD)
        hts = res0[m]["hts"][:hi - lo]
        htab1[lo:hi, 0:COUT1 + 2 * H1] = hts[:, 0:COUT1 + 2 * H1]
        adst1[lo:hi] = hts[:, COUT1 + 2 * H1:].copy().view(np.float32)

    # per-core adst in assigned-block order: [P, NB*H] (partition = dst local)
    def adst_input(adst_n, H):
        out = []
        for m in range(NCORES):
            a = np.zeros((NB, P, H), np.float32)
            for i in range(NB):
                g = asg[i, m]
                rows = adst_n[g * P:(g + 1) * P]
                a[i, :len(rows)] = rows
            out.append(np.ascontiguousarray(
                a.transpose(1, 0, 2).reshape(P, NB * H)).astype(BF16))
        return out

    # ---- layer 1 ----
    tkey = tuple(meta["Tm"])
    zb1 = bool(np.all(np.asarray(b1) == 0))
    key1 = (1, tkey, zb1)
    if key1 not in _prog_cache:
        _prog_cache[key1] = _build_layer(meta, 1, zb1)
    ncA = _prog_cache[key1]
    in_maps = _inputs_layer(meta, idx_all, dstl_pt, dstl_row, b1, 1)
    W2f = np.asarray(W2, np.float32)
    wasrc2 = W2f @ np.asarray(att_src2, np.float32).reshape(F1, 1)
    wadst2 = W2f @ np.asarray(att_dst2, np.float32).reshape(F1, 1)
    w2r_np = np.ascontiguousarray(
        np.concatenate([W2f, wasrc2, wadst2], axis=1).reshape(2, P, 66)
    ).astype(BF16)
    ident = np.eye(P, dtype=np.float32).astype(BF16)
    a1in = adst_input(adst1, H1)
    for m, mmap in enumerate(in_maps):
        mmap["w2r"] = w2r_np
        mmap["identT"] = ident
        mmap["htab"] = htab1
        mmap["adstT"] = a1in[m]
    resA = run_bass_kernel_spmd(ncA, in_maps, list(range(NCORES))).results

    # reassemble layer-2 table + adst2 from assigned-block outputs
    htab2 = np.zeros((NPAD, 128), BF16)
    adst2 = np.zeros((N + P, 1), np.float32)
    for m in range(NCORES):
        o2 = resA[m]["outT2"]
        for i in range(NB):
            g = asg[i, m]
            if g * P >= NPAD:
                continue
            hi = min(NPAD, (g + 1) * P) - g * P
            htab2[g * P:g * P + hi, 0:66] = o2[i * P:i * P + hi, 0:66]
            adst2[g * P:g * P + hi, 0] = (
                o2[i * P:i * P + hi, 66:68].copy().view(np.float32)[:, 0])

    # ---- layer 2 ----
    zb2 = bool(np.all(np.asarray(b2) == 0))
    key2 = (2, tkey, zb2)
    if key2 not in _prog_cache:
        _prog_cache[key2] = _build_layer(meta, 2, zb2)
    ncB = _prog_cache[key2]
    in_maps2 = _inputs_layer(meta, idx_all, dstl_pt, dstl_row, b2, 2)
    a2in = adst_input(adst2, 1)
    for m, mmap in enumerate(in_maps2):
        mmap["htab"] = htab2
        mmap["adstT"] = a2in[m]
    resB = run_bass_kernel_spmd(ncB, in_maps2, list(range(NCORES))).results

    out = np.zeros((N, F1), np.float32)
    for m in range(NCORES):
        o = resB[m]["outT"]
        for i in range(NB):
            g = asg[i, m]
            lo = g * P
            if lo >= N:
                continue
            hi = min(N, lo + P)
            out[lo:hi] = o[i * P:i * P + (hi - lo)].astype(np.float32)
    return out
